# revision 6
# baseline (speedup 1.0000x reference)
"""DeepseekV3 MLA flash-attention prefill kernel for 8 Trainium2 NeuronCores.

Sharding strategy (SPMD, one program for all 8 cores):
  Stage A (sequence-parallel, feature-major): core c computes the low-rank
    down-projections q_a = rms_norm(X @ Wqa), c_kv = rms_norm(ckv[:, :512]),
    k_pe(roped) for its 256 rows directly in transposed layout
    (lhsT = weight chunks, rhs = X^T).  The kv path goes first so its
    AllGather fires early and overlaps the q-chunk matmuls; the q AllGather
    then overlaps the K^T / V up-projections.  RoPE sin/cos tables are
    precomputed on the host (no on-device Sin/range-reduction).
  Stage B (head-parallel): core c owns heads {2c, 2c+1}: per head, q
    projections (Wqb + RoPE via duplicated rot columns) are computed per
    panel, then causal attention runs in (k, q) layout: softmax without
    max-subtraction, fully-masked k-blocks skipped, diagonal blocks masked
    with GpSimd affine_select.  The prob-sum is accumulated on the vector
    engine (PE only does one ones-matmul per panel), and the per-q
    normalization is folded into the attn^T eviction.  Each head's attn^T
    is exchanged with its own AllToAll so the first overlaps the second
    head.  SBUF for the K/V/ckv tiles is reserved ahead of stage A so their
    DMAs don't wait on stage-A anti-dependencies.
  Each core then computes its 256 output rows against the full Wo.
  Host concatenates.
"""

import sys

if '/opt/trn_rl_repo' not in sys.path:
    sys.path.insert(0, '/opt/trn_rl_repo')

import numpy as np
import ml_dtypes

import concourse.bass as bass
import concourse.mybir as mybir
import concourse.tile as tile
from concourse import bacc
from concourse.bass_utils import run_bass_kernel_spmd

f32 = mybir.dt.float32
f32r = mybir.dt.float32r
bf16 = mybir.dt.bfloat16
i32 = mybir.dt.int32
AF = mybir.ActivationFunctionType
ALU = mybir.AluOpType

NC_ = 8            # cores
S = 2048           # sequence
HID = 2048
QLR = 1536         # q lora rank
KVLR = 512         # kv lora rank
ROPE = 64
NOPE = 128
VD = 128
NH = 16
HPC = NH // NC_    # heads per core = 2
SL = S // NC_      # rows per core = 256
PANEL = 512        # q panel width
NPANEL = S // PANEL
NKB = S // 128     # 16 k blocks
QCH = QLR // 128   # 12
KCH = KVLR // 128  # 4
HCH = HID // 128   # 16
KVW = KVLR + ROPE  # 576 = kv + rope cols of wa
THETA = 10000.0
SM_SCALE = float((NOPE + ROPE) ** -0.5)

DT = bf16          # matmul dtype

_CACHE = {}


def build_program(dt):
    nc = bacc.Bacc("TRN2", target_bir_lowering=False, debug=False, num_devices=NC_)

    def din(name, shape):
        return nc.dram_tensor(name, shape, dt, kind="ExternalInput")

    # ---- external I/O (per-core data) ----
    x_t = din("x_t", [HID, SL])                 # X rows, transposed (hid-major)
    wa = din("wa", [HID, KVW + QLR])            # [Wkva(kv) | Wkva(pe, deint) | Wqa]
    wqb = din("wqb", [QLR, HPC * 256])          # [nope|pe_d|rot] per head
    wkvb_k = din("wkvb_k", [KVLR, HPC * NOPE])
    wkvb_v = din("wkvb_v", [KVLR, HPC * VD])
    wo = din("wo", [NH * VD, HID])
    ones_col = din("ones_col", [128, 1])
    ones_col32 = nc.dram_tensor("ones_col32", [128, 1], f32, kind="ExternalInput")
    ones_row = nc.dram_tensor("ones_row", [1, 128], f32, kind="ExternalInput")
    cos_loc = nc.dram_tensor("cos_loc", [ROPE, SL], f32, kind="ExternalInput")
    sin_loc = nc.dram_tensor("sin_loc", [ROPE, SL], f32, kind="ExternalInput")
    cos_all = din("cos_all", [ROPE, S])
    sin_all = din("sin_all", [ROPE, S])
    out_loc = nc.dram_tensor("out_loc", [SL, HID], f32, kind="ExternalOutput")

    AGKV_R = KVLR + ROPE   # 576 rows in the kv AllGather

    with tile.TileContext(nc) as tc:
        with tc.tile_pool(name="dram", bufs=1, space="DRAM") as dpool, \
             tc.tile_pool(name="consts", bufs=1) as cpool, \
             tc.tile_pool(name="sb_w", bufs=1) as wbp, \
             tc.tile_pool(name="sb_kv", bufs=1) as kvp:
            ag_in_kv = dpool.tile([AGKV_R, SL], dt)
            ag_out_kv = dpool.tile([NC_ * AGKV_R, SL], dt, addr_space="Shared")
            ag_in_q = dpool.tile([QCH * 128, SL], dt)
            ag_out_q = dpool.tile([NC_ * QCH * 128, SL], dt, addr_space="Shared")
            a2a_in = [dpool.tile([NC_ * VD, SL], dt, name=f"a2a_in{h}") for h in range(HPC)]
            a2a_out = [dpool.tile([NC_ * VD, SL], dt, name=f"a2a_out{h}") for h in range(HPC)]

            agkv_r = ag_out_kv.rearrange("(r c) q -> r c q", r=NC_)
            agq_r = ag_out_q.rearrange("(r c) q -> r c q", r=NC_)

            ocol = cpool.tile([128, 1], dt)
            ocol32 = cpool.tile([128, 1], f32)
            orow = cpool.tile([1, 128], f32r)
            cosl_t = cpool.tile([ROPE, SL], f32)
            sinl_t = cpool.tile([ROPE, SL], f32)
            cosa_t = cpool.tile([ROPE, S], dt)
            sina_t = cpool.tile([ROPE, S], dt)
            nc.sync.dma_start(out=ocol[:], in_=ones_col[:])
            nc.sync.dma_start(out=ocol32[:], in_=ones_col32[:])
            nc.sync.dma_start(out=orow[:], in_=ones_row[:].bitcast(f32r))
            nc.sync.dma_start(out=cosl_t[:], in_=cos_loc[:])
            nc.sync.dma_start(out=sinl_t[:], in_=sin_loc[:])
            nc.sync.dma_start(out=cosa_t[:], in_=cos_all[:])
            nc.sync.dma_start(out=sina_t[:], in_=sin_all[:])

            # stage-B K/V-side tiles reserved ahead of stage A so their DMAs /
            # writes don't wait on stage-A SBUF anti-dependencies
            kpe_g = kvp.tile([ROPE, S], dt, name="kpe_g")
            kT = [kvp.tile([128, S], dt, name=f"kT{h}") for h in range(HPC)]
            v_t = [kvp.tile([128, HPC * VD], dt, name=f"v_t{kb}") for kb in range(NKB)]
            ckv_g = [kvp.tile([128, S], dt, name=f"ckv_g{j}") for j in range(KCH)]

            # ================= Stage A: transposed down projections =================
            with tc.tile_pool(name="sa_x", bufs=1) as xp, \
                 tc.tile_pool(name="sa_wkv", bufs=1) as wpk, \
                 tc.tile_pool(name="sa_wq", bufs=1) as wpq, \
                 tc.tile_pool(name="sa_res", bufs=1) as rp, \
                 tc.tile_pool(name="sa_tmp", bufs=2) as tp, \
                 tc.tile_pool(name="sa_ps", bufs=3, space="PSUM") as pp, \
                 tc.tile_pool(name="sa_ps1", bufs=1, space="PSUM") as pp1:

                # interleave x chunk / kv-weight chunk loads so the first
                # accumulation can start as soon as the first pair lands
                xts = []
                wa_kv = []
                for k in range(HCH):
                    xt = xp.tile([128, SL], dt, name=f"xt{k}")
                    nc.sync.dma_start(out=xt[:], in_=x_t[128 * k:128 * (k + 1), :])
                    xts.append(xt)
                    wt = wpk.tile([128, KVW], dt, name=f"wAkv_{k}")
                    nc.sync.dma_start(out=wt[:], in_=wa[128 * k:128 * (k + 1), 0:KVW])
                    wa_kv.append(wt)
                # stage-B up-projection weights next in the sync queue
                wkk_t = []
                wkv_t = []
                for l in range(KCH):
                    t = wbp.tile([128, HPC * NOPE], dt, name=f"wkk_t{l}")
                    nc.sync.dma_start(out=t[:], in_=wkvb_k[128 * l:128 * (l + 1), :])
                    wkk_t.append(t)
                    t2 = wbp.tile([128, HPC * VD], dt, name=f"wkv_t{l}")
                    nc.sync.dma_start(out=t2[:], in_=wkvb_v[128 * l:128 * (l + 1), :])
                    wkv_t.append(t2)
                wa_q = []
                for k in range(HCH):
                    wt = wpq.tile([128, QLR], dt, name=f"wAq_{k}")
                    nc.sync.dma_start(out=wt[:], in_=wa[128 * k:128 * (k + 1), KVW:])
                    wa_q.append(wt)
                wqb_t = []
                for l in range(QCH):
                    t = wbp.tile([128, HPC * 256], dt, name=f"wqb_t{l}")
                    nc.sync.dma_start(out=t[:], in_=wqb[128 * l:128 * (l + 1), :])
                    wqb_t.append(t)

                def a_chunk(wts, o, c0, width, tag):
                    ps = pp.tile([width, SL], f32, name=f"ps_{tag}_{o}", tag="a_ps", bufs=3)
                    for hc in range(HCH):
                        nc.tensor.matmul(ps[:], wts[hc][:, c0:c0 + width], xts[hc][:],
                                         start=(hc == 0), stop=(hc == HCH - 1))
                    return ps

                # ---- kv chunks first: unblock the kv AllGather ASAP ----
                ssq_kv = pp1.tile([1, SL], f32, name="ssq_kv")
                kv_sb = []
                for o in range(KCH):
                    ps = a_chunk(wa_kv, o, 128 * o, 128, "kv")
                    sb = rp.tile([128, SL], f32, name=f"kv_sb{o}")
                    nc.vector.tensor_copy(sb[:], ps[:])
                    kv_sb.append(sb)
                    sq = tp.tile([128, SL], dt, name=f"sqk{o}", tag="sq", bufs=3)
                    nc.scalar.activation(sq[:], ps[:], AF.Square)
                    nc.tensor.matmul(ssq_kv[:], ocol[:], sq[:], start=(o == 0), stop=(o == KCH - 1))
                ps_pe = a_chunk(wa_kv, 0, KVLR, ROPE, "pe")

                # k_pe rope from host tables (transposed layout)
                krot = tp.tile([ROPE, SL], f32, name="krot", tag="krot", bufs=1)
                nc.vector.tensor_scalar(out=krot[0:32, :], in0=ps_pe[32:64, :], scalar1=-1.0, scalar2=None, op0=ALU.mult)
                nc.vector.tensor_copy(krot[32:64, :], ps_pe[0:32, :])
                kro = tp.tile([ROPE, SL], f32, name="kro", tag="kro", bufs=1)
                nc.vector.tensor_mul(kro[:], ps_pe[:], cosl_t[:])
                krs = tp.tile([ROPE, SL], f32, name="krs", tag="krs", bufs=1)
                nc.vector.tensor_mul(krs[:], krot[:], sinl_t[:])
                kfin = tp.tile([ROPE, SL], dt, name="kfin", tag="kfin", bufs=1)
                nc.vector.tensor_add(kfin[:], kro[:], krs[:])
                nc.scalar.dma_start(out=ag_in_kv[KVLR:KVLR + ROPE, :], in_=kfin[:])

                # kv rms scale + store
                ms_kv = tp.tile([1, SL], f32, name="ms_kv", tag="ms", bufs=2)
                nc.scalar.activation(ms_kv[:], ssq_kv[:], AF.Sqrt, scale=1.0 / KVLR)
                rkv = tp.tile([1, SL], f32, name="rkv", tag="rr", bufs=2)
                nc.vector.reciprocal_approx_fast(out=rkv[:], in_=ms_kv[:])
                rkv_r = tp.tile([1, SL], f32r, name="rkv_r", tag="rrr", bufs=2)
                with nc.allow_low_precision(reason="f32r rounding of rms scale"):
                    nc.vector.tensor_copy(rkv_r[:], rkv[:])
                bc_kv = pp1.tile([128, SL], f32, name="bc_kv", tag="bc", bufs=2)
                nc.tensor.matmul(bc_kv[:], orow[:], rkv_r[:], start=True, stop=True)
                for o in range(KCH):
                    sc = tp.tile([128, SL], dt, name=f"sck{o}", tag="sc", bufs=3)
                    nc.vector.tensor_mul(sc[:], kv_sb[o][:], bc_kv[:])
                    nc.scalar.dma_start(out=ag_in_kv[128 * o:128 * (o + 1), :], in_=sc[:])

                nc.gpsimd.collective_compute(
                    "AllGather", ALU.bypass,
                    replica_groups=[list(range(NC_))],
                    ins=[ag_in_kv[:]], outs=[ag_out_kv[:]],
                )

                # gathered kv reads ride the scalar queue (sync is busy with wa_q)
                for r in range(NC_):
                    nc.scalar.dma_start(out=kpe_g[:, SL * r:SL * (r + 1)],
                                        in_=agkv_r[r, KVLR:KVLR + ROPE, :])
                for j in range(KCH):
                    for r in range(NC_):
                        nc.scalar.dma_start(out=ckv_g[j][:, SL * r:SL * (r + 1)],
                                            in_=agkv_r[r, 128 * j:128 * (j + 1), :])

                # ---- q chunks (overlap the kv AllGather) ----
                ssq_q = pp1.tile([1, SL], f32, name="ssq_q")
                qa_sb = []
                for o in range(QCH):
                    ps = a_chunk(wa_q, o, 128 * o, 128, "q")
                    sb = rp.tile([128, SL], f32, name=f"qa_sb{o}")
                    nc.vector.tensor_copy(sb[:], ps[:])
                    qa_sb.append(sb)
                    sq = tp.tile([128, SL], dt, name=f"sqq{o}", tag="sq", bufs=3)
                    nc.scalar.activation(sq[:], ps[:], AF.Square)
                    nc.tensor.matmul(ssq_q[:], ocol[:], sq[:], start=(o == 0), stop=(o == QCH - 1))
                ms_q = tp.tile([1, SL], f32, name="ms_q", tag="ms", bufs=2)
                nc.scalar.activation(ms_q[:], ssq_q[:], AF.Sqrt, scale=1.0 / QLR)
                rq = tp.tile([1, SL], f32, name="rq", tag="rr", bufs=2)
                nc.vector.reciprocal_approx_fast(out=rq[:], in_=ms_q[:])
                rq_r = tp.tile([1, SL], f32r, name="rq_r", tag="rrr", bufs=2)
                with nc.allow_low_precision(reason="f32r rounding of rms scale"):
                    nc.vector.tensor_copy(rq_r[:], rq[:])
                bc_q = pp1.tile([128, SL], f32, name="bc_q", tag="bc", bufs=2)
                nc.tensor.matmul(bc_q[:], orow[:], rq_r[:], start=True, stop=True)
                for o in range(QCH):
                    sc = tp.tile([128, SL], dt, name=f"scq{o}", tag="sc", bufs=3)
                    nc.vector.tensor_mul(sc[:], qa_sb[o][:], bc_q[:])
                    nc.scalar.dma_start(out=ag_in_q[128 * o:128 * (o + 1), :], in_=sc[:])

                nc.gpsimd.collective_compute(
                    "AllGather", ALU.bypass,
                    replica_groups=[list(range(NC_))],
                    ins=[ag_in_q[:]], outs=[ag_out_q[:]],
                )

            # ================= Stage B: head-parallel attention =================
            with tc.tile_pool(name="sb_res", bufs=1) as rp, \
                 tc.tile_pool(name="sb_qa", bufs=2) as qap, \
                 tc.tile_pool(name="sb_tmp", bufs=2) as tp, \
                 tc.tile_pool(name="sb_pt", bufs=4) as ptp, \
                 tc.tile_pool(name="sb_wo", bufs=1) as wsp, \
                 tc.tile_pool(name="sb_ag", bufs=1) as agp, \
                 tc.tile_pool(name="sb_ps", bufs=2, space="PSUM") as pp, \
                 tc.tile_pool(name="sb_ps1", bufs=1, space="PSUM") as pp1:

                # K^T and V (both heads)
                for h in range(HPC):
                    for kc in range(S // 512):
                        ps = pp.tile([128, 512], f32, name=f"kt_ps{h}_{kc}", tag="mm_ps", bufs=2)
                        for l in range(KCH):
                            nc.tensor.matmul(ps[:], wkk_t[l][:, NOPE * h:NOPE * (h + 1)],
                                             ckv_g[l][:, 512 * kc:512 * (kc + 1)],
                                             start=(l == 0), stop=(l == KCH - 1))
                        nc.vector.tensor_copy(kT[h][:, 512 * kc:512 * (kc + 1)], ps[:])
                for kb in range(NKB):
                    ps = pp.tile([128, HPC * VD], f32, name=f"v_ps{kb}", tag="mm_ps", bufs=2)
                    for l in range(KCH):
                        nc.tensor.matmul(ps[:], ckv_g[l][:, 128 * kb:128 * (kb + 1)], wkv_t[l][:],
                                         start=(l == 0), stop=(l == KCH - 1))
                    nc.vector.tensor_copy(v_t[kb][:], ps[:])

                # ---- per head: q projections + attention, then AllToAll ----
                wo_map = {}
                att_g = {}
                for h in range(HPC):
                    qn_sb = {}
                    qp_sb = {}
                    for p in range(NPANEL):
                        qs = slice(PANEL * p, PANEL * (p + 1))
                        qa_p = []
                        for l in range(QCH):
                            t = qap.tile([128, PANEL], dt, name=f"qa_p{h}_{p}_{l}", tag=f"qa_p{l}", bufs=2)
                            for r in range(2):
                                nc.sync.dma_start(out=t[:, SL * r:SL * (r + 1)],
                                                  in_=agq_r[2 * p + r, 128 * l:128 * (l + 1), :])
                            qa_p.append(t)
                        hcol = 256 * h
                        ps_qn = pp.tile([128, PANEL], f32, name=f"qn_ps{h}_{p}", tag="mm_ps", bufs=2)
                        for l in range(QCH):
                            nc.tensor.matmul(ps_qn[:], wqb_t[l][:, hcol:hcol + NOPE], qa_p[l][:],
                                             start=(l == 0), stop=(l == QCH - 1))
                        ps_qr = pp.tile([128, PANEL], f32, name=f"qr_ps{h}_{p}", tag="mm_ps", bufs=2)
                        for l in range(QCH):
                            nc.tensor.matmul(ps_qr[:], wqb_t[l][:, hcol + NOPE:hcol + 256], qa_p[l][:],
                                             start=(l == 0), stop=(l == QCH - 1))
                        qn = rp.tile([128, PANEL], dt, name=f"qn_sb{h}_{p}", tag=f"qn{p}", bufs=1)
                        nc.vector.tensor_copy(qn[:], ps_qn[:])
                        qn_sb[p] = qn
                        qt1 = tp.tile([ROPE, PANEL], f32, name=f"qt1_{h}_{p}", tag="qt1", bufs=2)
                        nc.vector.tensor_mul(qt1[:], ps_qr[0:ROPE, :], cosa_t[:, qs])
                        qt2 = tp.tile([ROPE, PANEL], f32, name=f"qt2_{h}_{p}", tag="qt2", bufs=2)
                        nc.vector.tensor_mul(qt2[:], ps_qr[ROPE:2 * ROPE, :], sina_t[:, qs])
                        qp = rp.tile([ROPE, PANEL], dt, name=f"qp_sb{h}_{p}", tag=f"qp{p}", bufs=1)
                        nc.vector.tensor_add(qp[:], qt1[:], qt2[:])
                        qp_sb[p] = qp

                    if h == 0:
                        # Wo preload: issued here so the sync DMA queue serves the
                        # stage-A weight stream and the h0 q-panels first
                        for col in range(HID // 512):
                            for c in range(HCH):
                                t = wsp.tile([128, 512], dt, name=f"wo_s{c}_{col}", tag="wo_s", bufs=64)
                                nc.sync.dma_start(out=t[:], in_=wo[128 * c:128 * (c + 1), 512 * col:512 * (col + 1)])
                                wo_map[(c, col)] = t[:]

                    for p in range(NPANEL):
                        nkb = 4 * (p + 1)
                        ps_at = pp1.tile([128, PANEL], f32, name=f"at_ps{h}_{p}", tag="at_ps", bufs=2)
                        acc = tp.tile([128, PANEL], f32, name=f"acc{h}_{p}", tag="acc", bufs=2)
                        pts = {}

                        for kb in range(nkb):
                            ps_sc = pp.tile([128, PANEL], f32, name=f"sc_ps{h}_{p}_{kb}", tag="sc_ps", bufs=3)
                            nc.tensor.matmul(ps_sc[:], kT[h][:, 128 * kb:128 * (kb + 1)], qn_sb[p][:],
                                             start=True, stop=False)
                            nc.tensor.matmul(ps_sc[:], kpe_g[:, 128 * kb:128 * (kb + 1)], qp_sb[p][:],
                                             start=False, stop=True)
                            pt = ptp.tile([128, PANEL], dt, name=f"pt{h}_{p}_{kb}", tag="pt", bufs=4)
                            nc.scalar.activation(pt[:], ps_sc[:], AF.Exp, scale=SM_SCALE)
                            if kb >= 4 * p:
                                j = kb - 4 * p
                                nc.gpsimd.affine_select(
                                    out=pt[:], in_=pt[:],
                                    pattern=[[1, PANEL]],
                                    compare_op=ALU.is_ge,
                                    fill=0.0,
                                    base=-128 * j,
                                    channel_multiplier=-1)
                            pts[kb] = pt
                            # prob-sum accumulated on the vector engine
                            if kb == 0:
                                nc.vector.tensor_copy(acc[:], pt[:])
                            else:
                                nc.vector.tensor_add(acc[:], acc[:], pt[:])
                            nc.tensor.matmul(ps_at[:], v_t[kb][:, VD * h:VD * (h + 1)], pts[kb][:],
                                             start=(kb == 0), stop=(kb == nkb - 1))
                        ps_sum = pp1.tile([1, PANEL], f32, name=f"sum_ps{h}_{p}", tag="sm_bc", bufs=1)
                        nc.tensor.matmul(ps_sum[:], ocol32[:], acc[:], start=True, stop=True)
                        rec = tp.tile([1, PANEL], f32, name=f"rec{h}_{p}", tag="rec", bufs=2)
                        nc.vector.reciprocal_approx_fast(out=rec[:], in_=ps_sum[:])
                        rec_r = tp.tile([1, PANEL], f32r, name=f"rec_r{h}_{p}", tag="rec_r", bufs=2)
                        with nc.allow_low_precision(reason="f32r rounding of softmax recip"):
                            nc.vector.tensor_copy(rec_r[:], rec[:])
                        bc = pp1.tile([128, PANEL], f32, name=f"bc_ps{h}_{p}", tag="sm_bc", bufs=1)
                        nc.tensor.matmul(bc[:], orow[:], rec_r[:], start=True, stop=True)
                        bc_sb = tp.tile([128, PANEL], f32, name=f"bc_sb{h}_{p}", tag="bc_sb", bufs=2)
                        nc.vector.tensor_copy(bc_sb[:], bc[:])
                        at_p = tp.tile([128, PANEL], dt, name=f"at_p{h}_{p}", tag="at_p", bufs=2)
                        nc.vector.tensor_mul(at_p[:], ps_at[:], bc_sb[:])
                        for r in range(2):
                            j = 2 * p + r
                            nc.scalar.dma_start(
                                out=a2a_in[h][j * VD:(j + 1) * VD, :],
                                in_=at_p[:, SL * r:SL * (r + 1)])
                    nc.gpsimd.collective_compute(
                        "AllToAll", ALU.bypass,
                        replica_groups=[list(range(NC_))],
                        ins=[a2a_in[h][:]], outs=[a2a_out[h][:]],
                    )
                    for j in range(NC_):
                        c = 2 * j + h
                        t = agp.tile([128, SL], dt, name=f"att_g{c}")
                        nc.sync.dma_start(out=t[:], in_=a2a_out[h][128 * j:128 * (j + 1), :])
                        att_g[c] = t

                # ---- Wo: seq-parallel output projection ----
                for col in range(HID // 512):
                    for qb in range(SL // 128):
                        ps = pp.tile([128, 512], f32, name=f"o_ps{col}_{qb}", tag="mm_ps", bufs=2)
                        for c in range(HCH):
                            nc.tensor.matmul(ps[:], att_g[c][:, 128 * qb:128 * (qb + 1)], wo_map[(c, col)],
                                             start=(c == 0), stop=(c == HCH - 1))
                        osb = tp.tile([128, 512], f32, name=f"osb{col}_{qb}", tag="osb", bufs=3)
                        nc.vector.tensor_copy(osb[:], ps[:])
                        nc.sync.dma_start(out=out_loc[128 * qb:128 * (qb + 1), 512 * col:512 * (col + 1)], in_=osb[:])

    nc.compile()
    return nc


def _to_dt(a, dt):
    if dt == bf16:
        return np.ascontiguousarray(a.astype(ml_dtypes.bfloat16))
    return np.ascontiguousarray(a.astype(np.float32))


def _prepare_inputs(dt, hidden_states, position_ids, Wqa, qa_ln_w, Wqb, Wkva, kv_ln_w, Wkvb, Wo):
    perm = np.concatenate([np.arange(0, ROPE, 2), np.arange(1, ROPE, 2)])
    X = np.asarray(hidden_states, np.float32).reshape(S, HID)
    Wqa = np.asarray(Wqa, np.float32)
    Wkva = np.asarray(Wkva, np.float32)
    # kv cols first so stage A can load + compute the kv path before q
    wa = np.concatenate([Wkva[:, :KVLR], Wkva[:, KVLR:][:, perm], Wqa], axis=1)  # (2048, 2112)
    wqb_base = np.asarray(Wqb, np.float32) * np.asarray(qa_ln_w, np.float32)[:, None]
    wkvb_base = np.asarray(Wkvb, np.float32) * np.asarray(kv_ln_w, np.float32)[:, None]
    Wo = np.asarray(Wo, np.float32)

    head_blocks = []
    for h in range(NH):
        cols = wqb_base[:, 192 * h:192 * (h + 1)]
        nope = cols[:, :NOPE]
        pe_d = cols[:, NOPE:][:, perm]
        rot = np.concatenate([-pe_d[:, 32:], pe_d[:, :32]], axis=1)
        head_blocks.append(np.concatenate([nope, pe_d, rot], axis=1))  # (1536, 256)
    k_blocks = [wkvb_base[:, 256 * h:256 * h + NOPE] for h in range(NH)]
    v_blocks = [wkvb_base[:, 256 * h + NOPE:256 * (h + 1)] for h in range(NH)]

    # host-precomputed RoPE tables in deinterleaved layout: row d uses
    # inv_freq[d % 32], column t is position t
    pos = np.asarray(position_ids, np.float32).reshape(S)
    inv = (1.0 / (THETA ** (np.arange(0, ROPE, 2, dtype=np.float32) / ROPE))).astype(np.float32)
    invf = np.concatenate([inv, inv])                      # (64,)
    emb = invf[:, None] * pos[None, :]                     # (64, S)
    cos_np = np.cos(emb).astype(np.float32)
    sin_np = np.sin(emb).astype(np.float32)

    wa_d = _to_dt(wa, dt)
    wo_d = _to_dt(Wo, dt)
    ones_col_d = _to_dt(np.ones((128, 1), np.float32), dt)
    cos_all_d = _to_dt(cos_np, dt)
    sin_all_d = _to_dt(sin_np, dt)

    in_maps = []
    for c in range(NC_):
        rows = slice(SL * c, SL * (c + 1))
        in_maps.append({
            "x_t": _to_dt(X[rows, :].T, dt),
            "wa": wa_d,
            "wqb": _to_dt(np.concatenate([head_blocks[HPC * c + h] for h in range(HPC)], axis=1), dt),
            "wkvb_k": _to_dt(np.concatenate([k_blocks[HPC * c + h] for h in range(HPC)], axis=1), dt),
            "wkvb_v": _to_dt(np.concatenate([v_blocks[HPC * c + h] for h in range(HPC)], axis=1), dt),
            "wo": wo_d,
            "ones_col": ones_col_d,
            "ones_col32": np.ones((128, 1), np.float32),
            "ones_row": np.ones((1, 128), np.float32),
            "cos_loc": np.ascontiguousarray(cos_np[:, rows]),
            "sin_loc": np.ascontiguousarray(sin_np[:, rows]),
            "cos_all": cos_all_d,
            "sin_all": sin_all_d,
        })
    return in_maps


def run(inputs, trace=False, trace_cores=None, dt=None):
    dt = dt if dt is not None else DT
    key = ("nc", str(dt))
    if key not in _CACHE:
        _CACHE[key] = build_program(dt)
    nc = _CACHE[key]
    in_maps = _prepare_inputs(dt, **inputs)
    res = run_bass_kernel_spmd(nc, in_maps, list(range(NC_)), trace=trace,
                               trace_cores=trace_cores)
    out = np.concatenate([res.results[c]["out_loc"] for c in range(NC_)], axis=0)
    return out.reshape(1, S, HID), res


def kernel(**inputs) -> np.ndarray:
    out, _ = run(inputs, trace=False)
    return out


# revision 21
# speedup vs baseline: 1.0014x; 1.0014x over previous
"""DeepseekV3 MLA flash-attention prefill kernel for 8 Trainium2 NeuronCores.

Sharding strategy (SPMD, one program for all 8 cores):
  Stage A (sequence-parallel, feature-major): core c computes the low-rank
    down-projections q_a = rms_norm(X @ Wqa), c_kv = rms_norm(ckv[:, :512]),
    k_pe(roped) for its 256 rows directly in transposed layout
    (lhsT = weight chunks, rhs = X^T).  The kv path goes first so its
    AllGather fires early and overlaps the q-chunk matmuls; the q AllGather
    then overlaps the K^T / V up-projections.  RoPE sin/cos tables are
    precomputed on the host (no on-device Sin/range-reduction).
  Stage B (head-parallel): core c owns heads {2c, 2c+1}: per head, q
    projections (Wqb + RoPE via duplicated rot columns) are computed per
    panel, then causal attention runs in (k, q) layout: softmax without
    max-subtraction, fully-masked k-blocks skipped, diagonal blocks masked
    with GpSimd affine_select.  The prob-sum is accumulated on the vector
    engine (PE only does one ones-matmul per panel), and the per-q
    normalization is folded into the attn^T eviction.  Each head's attn^T
    is exchanged with its own AllToAll so the first overlaps the second
    head.  SBUF for the K/V/ckv tiles is reserved ahead of stage A so their
    DMAs don't wait on stage-A anti-dependencies.
  Each core then computes its 256 output rows against the full Wo.
  Host concatenates.
"""

import sys

if '/opt/trn_rl_repo' not in sys.path:
    sys.path.insert(0, '/opt/trn_rl_repo')

import numpy as np
import ml_dtypes

import concourse.bass as bass
import concourse.mybir as mybir
import concourse.tile as tile
from concourse import bacc
from concourse.bass_utils import run_bass_kernel_spmd

f32 = mybir.dt.float32
f32r = mybir.dt.float32r
bf16 = mybir.dt.bfloat16
i32 = mybir.dt.int32
AF = mybir.ActivationFunctionType
ALU = mybir.AluOpType

NC_ = 8            # cores
S = 2048           # sequence
HID = 2048
QLR = 1536         # q lora rank
KVLR = 512         # kv lora rank
ROPE = 64
NOPE = 128
VD = 128
NH = 16
HPC = NH // NC_    # heads per core = 2
SL = S // NC_      # rows per core = 256
PANEL = 512        # q panel width
NPANEL = S // PANEL
NKB = S // 128     # 16 k blocks
QCH = QLR // 128   # 12
KCH = KVLR // 128  # 4
HCH = HID // 128   # 16
KVW = KVLR + ROPE  # 576 = kv + rope cols of wa
THETA = 10000.0
SM_SCALE = float((NOPE + ROPE) ** -0.5)

DT = bf16          # matmul dtype

_CACHE = {}


def build_program(dt):
    nc = bacc.Bacc("TRN2", target_bir_lowering=False, debug=False, num_devices=NC_)

    def din(name, shape):
        return nc.dram_tensor(name, shape, dt, kind="ExternalInput")

    # ---- external I/O (per-core data) ----
    x_t = din("x_t", [HID, SL])                 # X rows, transposed (hid-major)
    wa = din("wa", [HID, KVW + QLR])            # [Wkva(kv) | Wkva(pe, deint) | Wqa]
    wqb = din("wqb", [QLR, HPC * 256])          # [nope|pe_d|rot] per head
    wkvb_k = din("wkvb_k", [KVLR, HPC * NOPE])
    wkvb_v = din("wkvb_v", [KVLR, HPC * VD])
    wo = din("wo", [NH * VD, HID])
    ones_col = din("ones_col", [128, 1])
    ones_col32 = nc.dram_tensor("ones_col32", [128, 1], f32, kind="ExternalInput")
    ones_row = nc.dram_tensor("ones_row", [1, 128], f32, kind="ExternalInput")
    cos_loc = nc.dram_tensor("cos_loc", [ROPE, SL], f32, kind="ExternalInput")
    sin_loc = nc.dram_tensor("sin_loc", [ROPE, SL], f32, kind="ExternalInput")
    cos_all = din("cos_all", [ROPE, S])
    sin_all = din("sin_all", [ROPE, S])
    out_loc = nc.dram_tensor("out_loc", [SL, HID], f32, kind="ExternalOutput")

    AGKV_R = KVLR + ROPE   # 576 rows in the kv AllGather

    with tile.TileContext(nc) as tc:
        with tc.tile_pool(name="dram", bufs=1, space="DRAM") as dpool, \
             tc.tile_pool(name="consts", bufs=1) as cpool, \
             tc.tile_pool(name="sb_w", bufs=1) as wbp, \
             tc.tile_pool(name="sb_kv", bufs=1) as kvp:
            dummy_in = dpool.tile([128, 1], dt)
            dummy_out = dpool.tile([NC_ * 128, 1], dt, addr_space="Shared")
            ag_in_kv = dpool.tile([AGKV_R, SL], dt)
            ag_out_kv = dpool.tile([NC_ * AGKV_R, SL], dt, addr_space="Shared")
            ag_in_q = dpool.tile([QCH * 128, SL], dt)
            ag_out_q = dpool.tile([NC_ * QCH * 128, SL], dt, addr_space="Shared")
            a2a_in = [dpool.tile([NC_ * VD, SL], dt, name=f"a2a_in{h}") for h in range(HPC)]
            a2a_out = [dpool.tile([NC_ * VD, SL], dt, name=f"a2a_out{h}") for h in range(HPC)]

            agkv_r = ag_out_kv.rearrange("(r c) q -> r c q", r=NC_)
            agq_r = ag_out_q.rearrange("(r c) q -> r c q", r=NC_)

            ocol = cpool.tile([128, 1], dt)
            ocol32 = cpool.tile([128, 1], f32)
            orow = cpool.tile([1, 128], f32r)
            cosa_t = cpool.tile([ROPE, S], dt)
            sina_t = cpool.tile([ROPE, S], dt)
            nc.sync.dma_start(out=ocol[:], in_=ones_col[:])
            nc.sync.dma_start(out=ocol32[:], in_=ones_col32[:])
            nc.sync.dma_start(out=orow[:], in_=ones_row[:].bitcast(f32r))

            # tiny warm-up collective: absorbs the ~11us CC-core first-call
            # latency and aligns rank start skew before the real AllGathers
            nc.scalar.dma_start(out=dummy_in[:], in_=ocol[:])
            nc.gpsimd.collective_compute(
                "AllGather", ALU.bypass,
                replica_groups=[list(range(NC_))],
                ins=[dummy_in[:]], outs=[dummy_out[:]],
            )

            # stage-B K/V-side tiles reserved ahead of stage A so their DMAs /
            # writes don't wait on stage-A SBUF anti-dependencies
            kpe_g = kvp.tile([ROPE, S], dt, name="kpe_g")
            kT = [kvp.tile([128, S], dt, name=f"kT{h}") for h in range(HPC)]
            v_t = [kvp.tile([128, HPC * VD], dt, name=f"v_t{kb}") for kb in range(NKB)]
            ckv_g = [kvp.tile([128, S], dt, name=f"ckv_g{j}") for j in range(KCH)]

            # ================= Stage A: transposed down projections =================
            with tc.tile_pool(name="sa_x", bufs=1) as xp, \
                 tc.tile_pool(name="sa_wkv", bufs=1) as wpk, \
                 tc.tile_pool(name="sa_wq", bufs=1) as wpq, \
                 tc.tile_pool(name="sa_res", bufs=1) as rp, \
                 tc.tile_pool(name="sa_tmp", bufs=2) as tp, \
                 tc.tile_pool(name="sa_ps", bufs=3, space="PSUM") as pp, \
                 tc.tile_pool(name="sa_ps1", bufs=1, space="PSUM") as pp1:

                cosl_t = rp.tile([ROPE, SL], f32, name="cosl_t")
                sinl_t = rp.tile([ROPE, SL], f32, name="sinl_t")
                nc.sync.dma_start(out=cosl_t[:], in_=cos_loc[:])
                nc.sync.dma_start(out=sinl_t[:], in_=sin_loc[:])

                # interleave x chunk / kv-weight chunk loads so the first
                # accumulation can start as soon as the first pair lands
                xts = []
                wa_kv = []
                for k in range(HCH):
                    xt = xp.tile([128, SL], dt, name=f"xt{k}")
                    nc.sync.dma_start(out=xt[:], in_=x_t[128 * k:128 * (k + 1), :])
                    xts.append(xt)
                    wt = wpk.tile([128, KVW], dt, name=f"wAkv_{k}")
                    nc.sync.dma_start(out=wt[:], in_=wa[128 * k:128 * (k + 1), 0:KVW])
                    wa_kv.append(wt)
                # stage-B up-projection weights next in the sync queue
                wkk_t = []
                wkv_t = []
                for l in range(KCH):
                    t = wbp.tile([128, HPC * NOPE], dt, name=f"wkk_t{l}")
                    nc.sync.dma_start(out=t[:], in_=wkvb_k[128 * l:128 * (l + 1), :])
                    wkk_t.append(t)
                    t2 = wbp.tile([128, HPC * VD], dt, name=f"wkv_t{l}")
                    nc.sync.dma_start(out=t2[:], in_=wkvb_v[128 * l:128 * (l + 1), :])
                    wkv_t.append(t2)
                wa_q = []
                for k in range(HCH):
                    wt = wpq.tile([128, QLR], dt, name=f"wAq_{k}")
                    nc.sync.dma_start(out=wt[:], in_=wa[128 * k:128 * (k + 1), KVW:])
                    wa_q.append(wt)
                wqb_t = []
                for l in range(QCH):
                    t = wbp.tile([128, HPC * 256], dt, name=f"wqb_t{l}")
                    nc.sync.dma_start(out=t[:], in_=wqb[128 * l:128 * (l + 1), :])
                    wqb_t.append(t)
                nc.sync.dma_start(out=cosa_t[:], in_=cos_all[:])
                nc.sync.dma_start(out=sina_t[:], in_=sin_all[:])

                def a_chunk(wts, o, c0, width, tag):
                    ps = pp.tile([width, SL], f32, name=f"ps_{tag}_{o}", tag="a_ps", bufs=3)
                    for hc in range(HCH):
                        nc.tensor.matmul(ps[:], wts[hc][:, c0:c0 + width], xts[hc][:],
                                         start=(hc == 0), stop=(hc == HCH - 1))
                    return ps

                # ---- k_pe + kv chunks first: unblock the kv AllGather ASAP ----
                ps_pe = a_chunk(wa_kv, 0, KVLR, ROPE, "pe")

                ssq_kv = pp1.tile([1, SL], f32, name="ssq_kv")
                kv_sb = []
                for o in range(KCH):
                    ps = a_chunk(wa_kv, o, 128 * o, 128, "kv")
                    sb = rp.tile([128, SL], f32, name=f"kv_sb{o}")
                    nc.vector.tensor_copy(sb[:], ps[:])
                    kv_sb.append(sb)
                    sq = tp.tile([128, SL], dt, name=f"sqk{o}", tag="sq", bufs=3)
                    nc.scalar.activation(sq[:], ps[:], AF.Square)
                    nc.tensor.matmul(ssq_kv[:], ocol[:], sq[:], start=(o == 0), stop=(o == KCH - 1))

                # k_pe rope from host tables (transposed layout)
                krot = tp.tile([ROPE, SL], f32, name="krot", tag="krot", bufs=1)
                nc.vector.tensor_scalar(out=krot[0:32, :], in0=ps_pe[32:64, :], scalar1=-1.0, scalar2=None, op0=ALU.mult)
                nc.vector.tensor_copy(krot[32:64, :], ps_pe[0:32, :])
                kro = tp.tile([ROPE, SL], f32, name="kro", tag="kro", bufs=1)
                nc.vector.tensor_mul(kro[:], ps_pe[:], cosl_t[:])
                krs = tp.tile([ROPE, SL], f32, name="krs", tag="krs", bufs=1)
                nc.vector.tensor_mul(krs[:], krot[:], sinl_t[:])
                kfin = tp.tile([ROPE, SL], dt, name="kfin", tag="kfin", bufs=1)
                nc.vector.tensor_add(kfin[:], kro[:], krs[:])
                nc.scalar.dma_start(out=ag_in_kv[KVLR:KVLR + ROPE, :], in_=kfin[:])

                # kv rms scale + store
                ms_kv = tp.tile([1, SL], f32, name="ms_kv", tag="ms", bufs=2)
                nc.scalar.activation(ms_kv[:], ssq_kv[:], AF.Sqrt, scale=1.0 / KVLR)
                rkv = tp.tile([1, SL], f32, name="rkv", tag="rr", bufs=2)
                nc.vector.reciprocal_approx_fast(out=rkv[:], in_=ms_kv[:])
                rkv_r = tp.tile([1, SL], f32r, name="rkv_r", tag="rrr", bufs=2)
                with nc.allow_low_precision(reason="f32r rounding of rms scale"):
                    nc.vector.tensor_copy(rkv_r[:], rkv[:])
                bc_kv = pp1.tile([128, SL], f32, name="bc_kv", tag="bc", bufs=2)
                nc.tensor.matmul(bc_kv[:], orow[:], rkv_r[:], start=True, stop=True)
                for o in range(KCH):
                    sc = tp.tile([128, SL], dt, name=f"sck{o}", tag="sc", bufs=3)
                    nc.vector.tensor_mul(sc[:], kv_sb[o][:], bc_kv[:])
                    nc.scalar.dma_start(out=ag_in_kv[128 * o:128 * (o + 1), :], in_=sc[:])

                nc.gpsimd.collective_compute(
                    "AllGather", ALU.bypass,
                    replica_groups=[list(range(NC_))],
                    ins=[ag_in_kv[:]], outs=[ag_out_kv[:]],
                )

                # ---- q chunks (overlap the kv AllGather) ----
                ssq_q = pp1.tile([1, SL], f32, name="ssq_q")
                qa_sb = []
                for o in range(QCH):
                    ps = a_chunk(wa_q, o, 128 * o, 128, "q")
                    sb = rp.tile([128, SL], f32, name=f"qa_sb{o}")
                    nc.vector.tensor_copy(sb[:], ps[:])
                    qa_sb.append(sb)
                    sq = tp.tile([128, SL], dt, name=f"sqq{o}", tag="sq", bufs=3)
                    nc.scalar.activation(sq[:], ps[:], AF.Square)
                    nc.tensor.matmul(ssq_q[:], ocol[:], sq[:], start=(o == 0), stop=(o == QCH - 1))
                ms_q = tp.tile([1, SL], f32, name="ms_q", tag="ms", bufs=2)
                nc.scalar.activation(ms_q[:], ssq_q[:], AF.Sqrt, scale=1.0 / QLR)
                rq = tp.tile([1, SL], f32, name="rq", tag="rr", bufs=2)
                nc.vector.reciprocal_approx_fast(out=rq[:], in_=ms_q[:])
                rq_r = tp.tile([1, SL], f32r, name="rq_r", tag="rrr", bufs=2)
                with nc.allow_low_precision(reason="f32r rounding of rms scale"):
                    nc.vector.tensor_copy(rq_r[:], rq[:])
                bc_q = pp1.tile([128, SL], f32, name="bc_q", tag="bc", bufs=2)
                nc.tensor.matmul(bc_q[:], orow[:], rq_r[:], start=True, stop=True)
                for o in range(QCH):
                    sc = tp.tile([128, SL], dt, name=f"scq{o}", tag="sc", bufs=3)
                    nc.vector.tensor_mul(sc[:], qa_sb[o][:], bc_q[:])
                    nc.scalar.dma_start(out=ag_in_q[128 * o:128 * (o + 1), :], in_=sc[:])

                nc.gpsimd.collective_compute(
                    "AllGather", ALU.bypass,
                    replica_groups=[list(range(NC_))],
                    ins=[ag_in_q[:]], outs=[ag_out_q[:]],
                )

                # gathered kv reads ride the scalar queue AFTER all stage-A
                # scalar work (the wait on the kv AllGather must not block
                # the q-path squares / ag_in_q writes)
                for r in range(NC_):
                    nc.scalar.dma_start(out=kpe_g[:, SL * r:SL * (r + 1)],
                                        in_=agkv_r[r, KVLR:KVLR + ROPE, :])
                for j in range(KCH):
                    for r in range(NC_):
                        nc.scalar.dma_start(out=ckv_g[j][:, SL * r:SL * (r + 1)],
                                            in_=agkv_r[r, 128 * j:128 * (j + 1), :])

            # ================= Stage B: head-parallel attention =================
            with tc.tile_pool(name="sb_res", bufs=1) as rp, \
                 tc.tile_pool(name="sb_qa", bufs=2) as qap, \
                 tc.tile_pool(name="sb_tmp", bufs=2) as tp, \
                 tc.tile_pool(name="sb_pt", bufs=4) as ptp, \
                 tc.tile_pool(name="sb_wo", bufs=1) as wsp, \
                 tc.tile_pool(name="sb_ag", bufs=1) as agp, \
                 tc.tile_pool(name="sb_ps", bufs=2, space="PSUM") as pp, \
                 tc.tile_pool(name="sb_ps1", bufs=1, space="PSUM") as pp1:

                # K^T and V (both heads)
                for h in range(HPC):
                    for kc in range(S // 512):
                        ps = pp.tile([128, 512], f32, name=f"kt_ps{h}_{kc}", tag="mm_ps", bufs=2)
                        for l in range(KCH):
                            nc.tensor.matmul(ps[:], wkk_t[l][:, NOPE * h:NOPE * (h + 1)],
                                             ckv_g[l][:, 512 * kc:512 * (kc + 1)],
                                             start=(l == 0), stop=(l == KCH - 1))
                        nc.vector.tensor_copy(kT[h][:, 512 * kc:512 * (kc + 1)], ps[:])
                for kb in range(NKB):
                    ps = pp.tile([128, HPC * VD], f32, name=f"v_ps{kb}", tag="mm_ps", bufs=2)
                    for l in range(KCH):
                        nc.tensor.matmul(ps[:], ckv_g[l][:, 128 * kb:128 * (kb + 1)], wkv_t[l][:],
                                         start=(l == 0), stop=(l == KCH - 1))
                    nc.vector.tensor_copy(v_t[kb][:], ps[:])

                # ---- per head: q projections + attention, then AllToAll ----
                wo_map = {}
                att_gh = [agp.tile([128, NC_ * SL], dt, name=f"att_gh{h}") for h in range(HPC)]
                wo1_sb = {}
                for h in range(HPC):
                    qn_sb = {}
                    qp_sb = {}
                    for p in range(NPANEL):
                        qs = slice(PANEL * p, PANEL * (p + 1))
                        qa_p = []
                        for l in range(QCH):
                            t = qap.tile([128, PANEL], dt, name=f"qa_p{h}_{p}_{l}", tag=f"qa_p{l}", bufs=2)
                            for r in range(2):
                                nc.sync.dma_start(out=t[:, SL * r:SL * (r + 1)],
                                                  in_=agq_r[2 * p + r, 128 * l:128 * (l + 1), :])
                            qa_p.append(t)
                        hcol = 256 * h
                        ps_qn = pp.tile([128, PANEL], f32, name=f"qn_ps{h}_{p}", tag="mm_ps", bufs=2)
                        for l in range(QCH):
                            nc.tensor.matmul(ps_qn[:], wqb_t[l][:, hcol:hcol + NOPE], qa_p[l][:],
                                             start=(l == 0), stop=(l == QCH - 1))
                        ps_qr = pp.tile([128, PANEL], f32, name=f"qr_ps{h}_{p}", tag="mm_ps", bufs=2)
                        for l in range(QCH):
                            nc.tensor.matmul(ps_qr[:], wqb_t[l][:, hcol + NOPE:hcol + 256], qa_p[l][:],
                                             start=(l == 0), stop=(l == QCH - 1))
                        qn = rp.tile([128, PANEL], dt, name=f"qn_sb{h}_{p}", tag=f"qn{p}", bufs=1)
                        nc.vector.tensor_copy(qn[:], ps_qn[:])
                        qn_sb[p] = qn
                        qt1 = tp.tile([ROPE, PANEL], f32, name=f"qt1_{h}_{p}", tag="qt1", bufs=1)
                        nc.vector.tensor_mul(qt1[:], ps_qr[0:ROPE, :], cosa_t[:, qs])
                        qt2 = tp.tile([ROPE, PANEL], f32, name=f"qt2_{h}_{p}", tag="qt2", bufs=1)
                        nc.vector.tensor_mul(qt2[:], ps_qr[ROPE:2 * ROPE, :], sina_t[:, qs])
                        qp = rp.tile([ROPE, PANEL], dt, name=f"qp_sb{h}_{p}", tag=f"qp{p}", bufs=1)
                        nc.vector.tensor_add(qp[:], qt1[:], qt2[:])
                        qp_sb[p] = qp

                    if h == 0:
                        # Wo preload: issued here so the sync DMA queue serves the
                        # stage-A weight stream and the h0 q-panels first
                        for col in range(HID // 512):
                            for c in range(HCH):
                                t = wsp.tile([128, 512], dt, name=f"wo_s{c}_{col}", tag="wo_s", bufs=64)
                                nc.sync.dma_start(out=t[:], in_=wo[128 * c:128 * (c + 1), 512 * col:512 * (col + 1)])
                                wo_map[(c, col)] = t[:]

                    for p in range(NPANEL):
                        if h == 1:
                            # Wo pass 1: the h0-half of the output projection,
                            # hidden under attention-h1 (needs only att_gh[0])
                            for t in (2 * p, 2 * p + 1):
                                col, qb = t // 2, t % 2
                                ps_w = pp.tile([128, 512], f32, name=f"o1_ps{t}", tag="mm_ps", bufs=2)
                                for j in range(NC_):
                                    nc.tensor.matmul(
                                        ps_w[:],
                                        att_gh[0][:, j * SL + 128 * qb:j * SL + 128 * (qb + 1)],
                                        wo_map[(2 * j, col)],
                                        start=(j == 0), stop=(j == NC_ - 1))
                                w1 = rp.tile([128, 512], dt, name=f"wo1_sb{t}", tag=f"wo1_{t}", bufs=1)
                                nc.vector.tensor_copy(w1[:], ps_w[:])
                                wo1_sb[t] = w1
                        nkb = 4 * (p + 1)
                        ps_at = pp1.tile([128, PANEL], f32, name=f"at_ps{h}_{p}", tag="at_ps", bufs=2)
                        acc = tp.tile([128, PANEL], f32, name=f"acc{h}_{p}", tag="acc", bufs=2)
                        pts = {}

                        for kb in range(nkb):
                            ps_sc = pp.tile([128, PANEL], f32, name=f"sc_ps{h}_{p}_{kb}", tag="sc_ps", bufs=3)
                            nc.tensor.matmul(ps_sc[:], kT[h][:, 128 * kb:128 * (kb + 1)], qn_sb[p][:],
                                             start=True, stop=False)
                            nc.tensor.matmul(ps_sc[:], kpe_g[:, 128 * kb:128 * (kb + 1)], qp_sb[p][:],
                                             start=False, stop=True)
                            pt = ptp.tile([128, PANEL], dt, name=f"pt{h}_{p}_{kb}", tag="pt", bufs=4)
                            nc.scalar.activation(pt[:], ps_sc[:], AF.Exp, scale=SM_SCALE)
                            if kb >= 4 * p:
                                j = kb - 4 * p
                                nc.gpsimd.affine_select(
                                    out=pt[:], in_=pt[:],
                                    pattern=[[1, PANEL]],
                                    compare_op=ALU.is_ge,
                                    fill=0.0,
                                    base=-128 * j,
                                    channel_multiplier=-1)
                            pts[kb] = pt
                            # prob-sum accumulated on the vector engine
                            if kb == 0:
                                nc.vector.tensor_copy(acc[:], pt[:])
                            else:
                                nc.vector.tensor_add(acc[:], acc[:], pt[:])
                            nc.tensor.matmul(ps_at[:], v_t[kb][:, VD * h:VD * (h + 1)], pts[kb][:],
                                             start=(kb == 0), stop=(kb == nkb - 1))
                        ps_sum = pp1.tile([1, PANEL], f32, name=f"sum_ps{h}_{p}", tag="sm_bc", bufs=1)
                        nc.tensor.matmul(ps_sum[:], ocol32[:], acc[:], start=True, stop=True)
                        rec = tp.tile([1, PANEL], f32, name=f"rec{h}_{p}", tag="rec", bufs=2)
                        nc.vector.reciprocal_approx_fast(out=rec[:], in_=ps_sum[:])
                        rec_r = tp.tile([1, PANEL], f32r, name=f"rec_r{h}_{p}", tag="rec_r", bufs=2)
                        with nc.allow_low_precision(reason="f32r rounding of softmax recip"):
                            nc.vector.tensor_copy(rec_r[:], rec[:])
                        bc = pp1.tile([128, PANEL], f32, name=f"bc_ps{h}_{p}", tag="sm_bc", bufs=1)
                        nc.tensor.matmul(bc[:], orow[:], rec_r[:], start=True, stop=True)
                        bc_sb = tp.tile([128, PANEL], f32, name=f"bc_sb{h}_{p}", tag="bc_sb", bufs=2)
                        nc.vector.tensor_copy(bc_sb[:], bc[:])
                        at_p = tp.tile([128, PANEL], dt, name=f"at_p{h}_{p}", tag="at_p", bufs=2)
                        nc.vector.tensor_mul(at_p[:], ps_at[:], bc_sb[:])
                        for r in range(2):
                            j = 2 * p + r
                            nc.scalar.dma_start(
                                out=a2a_in[h][j * VD:(j + 1) * VD, :],
                                in_=at_p[:, SL * r:SL * (r + 1)])
                    nc.gpsimd.collective_compute(
                        "AllToAll", ALU.bypass,
                        replica_groups=[list(range(NC_))],
                        ins=[a2a_in[h][:]], outs=[a2a_out[h][:]],
                    )
                    # gathered attn reads on gpsimd: the wait on the AllToAll
                    # must not block the sync/scalar queues
                    for j in range(NC_):
                        nc.gpsimd.dma_start(out=att_gh[h][:, SL * j:SL * (j + 1)],
                                            in_=a2a_out[h][128 * j:128 * (j + 1), :])

                # ---- Wo pass 2: h1-half + combine ----
                for t in range(8):
                    col, qb = t // 2, t % 2
                    ps = pp.tile([128, 512], f32, name=f"o_ps{t}", tag="mm_ps", bufs=2)
                    for j in range(NC_):
                        nc.tensor.matmul(
                            ps[:],
                            att_gh[1][:, j * SL + 128 * qb:j * SL + 128 * (qb + 1)],
                            wo_map[(2 * j + 1, col)],
                            start=(j == 0), stop=(j == NC_ - 1))
                    osb = tp.tile([128, 512], f32, name=f"osb{t}", tag="osb", bufs=2)
                    nc.vector.tensor_add(osb[:], ps[:], wo1_sb[t][:])
                    nc.sync.dma_start(out=out_loc[128 * qb:128 * (qb + 1), 512 * col:512 * (col + 1)], in_=osb[:])

    nc.compile()
    return nc


def _to_dt(a, dt):
    if dt == bf16:
        return np.ascontiguousarray(a.astype(ml_dtypes.bfloat16))
    return np.ascontiguousarray(a.astype(np.float32))


def _prepare_inputs(dt, hidden_states, position_ids, Wqa, qa_ln_w, Wqb, Wkva, kv_ln_w, Wkvb, Wo):
    perm = np.concatenate([np.arange(0, ROPE, 2), np.arange(1, ROPE, 2)])
    X = np.asarray(hidden_states, np.float32).reshape(S, HID)
    Wqa = np.asarray(Wqa, np.float32)
    Wkva = np.asarray(Wkva, np.float32)
    # kv cols first so stage A can load + compute the kv path before q
    wa = np.concatenate([Wkva[:, :KVLR], Wkva[:, KVLR:][:, perm], Wqa], axis=1)  # (2048, 2112)
    wqb_base = np.asarray(Wqb, np.float32) * np.asarray(qa_ln_w, np.float32)[:, None]
    wkvb_base = np.asarray(Wkvb, np.float32) * np.asarray(kv_ln_w, np.float32)[:, None]
    Wo = np.asarray(Wo, np.float32)

    head_blocks = []
    for h in range(NH):
        cols = wqb_base[:, 192 * h:192 * (h + 1)]
        nope = cols[:, :NOPE]
        pe_d = cols[:, NOPE:][:, perm]
        rot = np.concatenate([-pe_d[:, 32:], pe_d[:, :32]], axis=1)
        head_blocks.append(np.concatenate([nope, pe_d, rot], axis=1))  # (1536, 256)
    k_blocks = [wkvb_base[:, 256 * h:256 * h + NOPE] for h in range(NH)]
    v_blocks = [wkvb_base[:, 256 * h + NOPE:256 * (h + 1)] for h in range(NH)]

    # host-precomputed RoPE tables in deinterleaved layout: row d uses
    # inv_freq[d % 32], column t is position t
    pos = np.asarray(position_ids, np.float32).reshape(S)
    inv = (1.0 / (THETA ** (np.arange(0, ROPE, 2, dtype=np.float32) / ROPE))).astype(np.float32)
    invf = np.concatenate([inv, inv])                      # (64,)
    emb = invf[:, None] * pos[None, :]                     # (64, S)
    cos_np = np.cos(emb).astype(np.float32)
    sin_np = np.sin(emb).astype(np.float32)

    wa_d = _to_dt(wa, dt)
    wo_d = _to_dt(Wo, dt)
    ones_col_d = _to_dt(np.ones((128, 1), np.float32), dt)
    cos_all_d = _to_dt(cos_np, dt)
    sin_all_d = _to_dt(sin_np, dt)

    in_maps = []
    for c in range(NC_):
        rows = slice(SL * c, SL * (c + 1))
        in_maps.append({
            "x_t": _to_dt(X[rows, :].T, dt),
            "wa": wa_d,
            "wqb": _to_dt(np.concatenate([head_blocks[HPC * c + h] for h in range(HPC)], axis=1), dt),
            "wkvb_k": _to_dt(np.concatenate([k_blocks[HPC * c + h] for h in range(HPC)], axis=1), dt),
            "wkvb_v": _to_dt(np.concatenate([v_blocks[HPC * c + h] for h in range(HPC)], axis=1), dt),
            "wo": wo_d,
            "ones_col": ones_col_d,
            "ones_col32": np.ones((128, 1), np.float32),
            "ones_row": np.ones((1, 128), np.float32),
            "cos_loc": np.ascontiguousarray(cos_np[:, rows]),
            "sin_loc": np.ascontiguousarray(sin_np[:, rows]),
            "cos_all": cos_all_d,
            "sin_all": sin_all_d,
        })
    return in_maps


def run(inputs, trace=False, trace_cores=None, dt=None):
    dt = dt if dt is not None else DT
    key = ("nc", str(dt))
    if key not in _CACHE:
        _CACHE[key] = build_program(dt)
    nc = _CACHE[key]
    in_maps = _prepare_inputs(dt, **inputs)
    res = run_bass_kernel_spmd(nc, in_maps, list(range(NC_)), trace=trace,
                               trace_cores=trace_cores)
    out = np.concatenate([res.results[c]["out_loc"] for c in range(NC_)], axis=0)
    return out.reshape(1, S, HID), res


def kernel(**inputs) -> np.ndarray:
    out, _ = run(inputs, trace=False)
    return out


# revision 38
# speedup vs baseline: 1.0351x; 1.0337x over previous
"""DeepseekV3 MLA flash-attention prefill kernel for 8 Trainium2 NeuronCores.

Sharding strategy (SPMD, one program for all 8 cores):
  Stage A (sequence-parallel, feature-major): core c computes the low-rank
    down-projections q_a = rms_norm(X @ Wqa), c_kv = rms_norm(ckv[:, :512]),
    k_pe(roped) for its 256 rows directly in transposed layout
    (lhsT = weight chunks, rhs = X^T).  The kv path goes first so its
    AllGather fires early and overlaps the q-chunk matmuls; the q AllGather
    then overlaps the K^T / V up-projections.  RoPE sin/cos tables are
    precomputed on the host (no on-device Sin/range-reduction).
  Stage B (head-parallel): core c owns heads {2c, 2c+1}: per head, q
    projections (Wqb + RoPE via duplicated rot columns) are computed per
    panel, then causal attention runs in (k, q) layout: softmax without
    max-subtraction, fully-masked k-blocks skipped, diagonal blocks masked
    with GpSimd affine_select.  The prob-sum is accumulated on the vector
    engine (PE only does one ones-matmul per panel), and the per-q
    normalization is folded into the attn^T eviction.  Each head's attn^T
    is exchanged with its own AllToAll so the first overlaps the second
    head.  SBUF for the K/V/ckv tiles is reserved ahead of stage A so their
    DMAs don't wait on stage-A anti-dependencies.
  Each core then computes its 256 output rows against the full Wo.
  Host concatenates.
"""

import sys

if '/opt/trn_rl_repo' not in sys.path:
    sys.path.insert(0, '/opt/trn_rl_repo')

import numpy as np
import ml_dtypes

import concourse.bass as bass
import concourse.mybir as mybir
import concourse.tile as tile
from concourse import bacc
from concourse.bass_utils import run_bass_kernel_spmd

f32 = mybir.dt.float32
f32r = mybir.dt.float32r
bf16 = mybir.dt.bfloat16
i32 = mybir.dt.int32
AF = mybir.ActivationFunctionType
ALU = mybir.AluOpType

NC_ = 8            # cores
S = 2048           # sequence
HID = 2048
QLR = 1536         # q lora rank
KVLR = 512         # kv lora rank
ROPE = 64
NOPE = 128
VD = 128
NH = 16
HPC = NH // NC_    # heads per core = 2
SL = S // NC_      # rows per core = 256
PANEL = 512        # q panel width
NPANEL = S // PANEL
NKB = S // 128     # 16 k blocks
QCH = QLR // 128   # 12
KCH = KVLR // 128  # 4
HCH = HID // 128   # 16
KVW = KVLR + ROPE  # 576 = kv + rope cols of wa
THETA = 10000.0
SM_SCALE = float((NOPE + ROPE) ** -0.5)

DT = bf16          # matmul dtype

_CACHE = {}


def build_program(dt):
    nc = bacc.Bacc("TRN2", target_bir_lowering=False, debug=False, num_devices=NC_)

    def din(name, shape):
        return nc.dram_tensor(name, shape, dt, kind="ExternalInput")

    # ---- external I/O (per-core data) ----
    # [X^T | Wkva(kv) | Wkva(pe, deint) | Wqa] merged so each 128-row chunk
    # loads with one fat DMA (4.7KB per partition line)
    XWAW = SL + KVW + QLR
    xwa = din("xwa", [HID, XWAW])
    wqb = din("wqb", [QLR, HPC * 256])          # [nope|pe_d|rot] per head
    wkvb_k = din("wkvb_k", [KVLR, HPC * NOPE])
    wkvb_v = din("wkvb_v", [KVLR, HPC * VD])
    wo = din("wo", [NH * VD, HID])
    ones_col = din("ones_col", [128, 1])
    ones_col32 = nc.dram_tensor("ones_col32", [128, 1], f32, kind="ExternalInput")
    ones_row = nc.dram_tensor("ones_row", [1, 128], f32, kind="ExternalInput")
    cos_loc = nc.dram_tensor("cos_loc", [ROPE, SL], f32, kind="ExternalInput")
    sin_loc = nc.dram_tensor("sin_loc", [ROPE, SL], f32, kind="ExternalInput")
    cos_all = din("cos_all", [ROPE, S])
    sin_all = din("sin_all", [ROPE, S])
    out_loc = nc.dram_tensor("out_loc", [SL, HID], f32, kind="ExternalOutput")

    AGKV_R = KVLR + ROPE   # 576 rows in the kv AllGather

    with tile.TileContext(nc) as tc:
        with tc.tile_pool(name="dram", bufs=1, space="DRAM") as dpool, \
             tc.tile_pool(name="consts", bufs=1) as cpool, \
             tc.tile_pool(name="sb_w", bufs=1) as wbp, \
             tc.tile_pool(name="sb_kv", bufs=1) as kvp:
            dummy_in = dpool.tile([128, 1], dt)
            dummy_out = dpool.tile([NC_ * 128, 1], dt, addr_space="Shared")
            ag_in_kv = dpool.tile([AGKV_R, SL], dt)
            ag_out_kv = dpool.tile([NC_ * AGKV_R, SL], dt, addr_space="Shared")
            ag_in_q = dpool.tile([QCH * 128, SL], dt)
            ag_out_q = dpool.tile([NC_ * QCH * 128, SL], dt, addr_space="Shared")
            a2a_in = [dpool.tile([NC_ * VD, SL], dt, name=f"a2a_in{h}") for h in range(HPC)]
            a2a_out = [dpool.tile([NC_ * VD, SL], dt, name=f"a2a_out{h}") for h in range(HPC)]

            agkv_r = ag_out_kv.rearrange("(r c) q -> r c q", r=NC_)
            agq_r = ag_out_q.rearrange("(r c) q -> r c q", r=NC_)

            ocol = cpool.tile([128, 1], dt)
            ocol32 = cpool.tile([128, 1], f32)
            orow = cpool.tile([1, 128], f32r)
            cosa_t = cpool.tile([ROPE, S], dt)
            sina_t = cpool.tile([ROPE, S], dt)
            nc.sync.dma_start(out=ocol[:], in_=ones_col[:])
            nc.sync.dma_start(out=ocol32[:], in_=ones_col32[:])
            nc.sync.dma_start(out=orow[:], in_=ones_row[:].bitcast(f32r))

            # tiny warm-up collective: absorbs the ~11us CC-core first-call
            # latency and aligns rank start skew before the real AllGathers
            nc.scalar.dma_start(out=dummy_in[:], in_=ocol[:])
            nc.gpsimd.collective_compute(
                "AllGather", ALU.bypass,
                replica_groups=[list(range(NC_))],
                ins=[dummy_in[:]], outs=[dummy_out[:]],
            )

            # stage-B K/V-side tiles reserved ahead of stage A so their DMAs /
            # writes don't wait on stage-A SBUF anti-dependencies
            kpe_g = kvp.tile([ROPE, S], dt, name="kpe_g")
            kT = [kvp.tile([128, S], dt, name=f"kT{h}") for h in range(HPC)]
            v_t = [kvp.tile([128, HPC * VD], dt, name=f"v_t{kb}") for kb in range(NKB)]

            # ================= Stage A: transposed down projections =================
            with tc.tile_pool(name="sa_x", bufs=1) as xp, \
                 tc.tile_pool(name="sa_res", bufs=1) as rp, \
                 tc.tile_pool(name="sa_tmp", bufs=2) as tp, \
                 tc.tile_pool(name="sa_ps", bufs=3, space="PSUM") as pp, \
                 tc.tile_pool(name="sa_ps1", bufs=1, space="PSUM") as pp1:

                cosl_t = rp.tile([ROPE, SL], f32, name="cosl_t")
                sinl_t = rp.tile([ROPE, SL], f32, name="sinl_t")
                nc.sync.dma_start(out=cosl_t[:], in_=cos_loc[:])
                nc.sync.dma_start(out=sinl_t[:], in_=sin_loc[:])

                # merged x/weight chunk loads: one fat DMA per 128-row chunk
                xwa_t = []
                for k in range(HCH):
                    t = xp.tile([128, XWAW], dt, name=f"xwa_{k}")
                    nc.sync.dma_start(out=t[:], in_=xwa[128 * k:128 * (k + 1), :])
                    xwa_t.append(t)
                xts = [t[:, 0:SL] for t in xwa_t]
                # stage-B up-projection weights next in the sync queue
                wkk_t = []
                wkv_t = []
                for l in range(KCH):
                    t = wbp.tile([128, HPC * NOPE], dt, name=f"wkk_t{l}")
                    nc.sync.dma_start(out=t[:], in_=wkvb_k[128 * l:128 * (l + 1), :])
                    wkk_t.append(t)
                    t2 = wbp.tile([128, HPC * VD], dt, name=f"wkv_t{l}")
                    nc.sync.dma_start(out=t2[:], in_=wkvb_v[128 * l:128 * (l + 1), :])
                    wkv_t.append(t2)
                wqb_t = []
                for l in range(QCH):
                    t = wbp.tile([128, HPC * 256], dt, name=f"wqb_t{l}")
                    nc.sync.dma_start(out=t[:], in_=wqb[128 * l:128 * (l + 1), :])
                    wqb_t.append(t)
                nc.sync.dma_start(out=cosa_t[:], in_=cos_all[:])
                nc.sync.dma_start(out=sina_t[:], in_=sin_all[:])

                def a_chunk(c0, width, tag):
                    ps = pp.tile([width, SL], f32, name=f"ps_{tag}", tag="a_ps", bufs=3)
                    for hc in range(HCH):
                        nc.tensor.matmul(ps[:], xwa_t[hc][:, SL + c0:SL + c0 + width], xts[hc],
                                         start=(hc == 0), stop=(hc == HCH - 1))
                    return ps

                # ---- k_pe + kv chunks first: unblock the kv AllGather ASAP ----
                ps_pe = a_chunk(KVLR, ROPE, "pe")

                ssq_kv = pp1.tile([1, SL], f32, name="ssq_kv")
                kv_sb = []
                for o in range(KCH):
                    ps = a_chunk(128 * o, 128, f"kv{o}")
                    sb = rp.tile([128, SL], f32, name=f"kv_sb{o}")
                    nc.vector.tensor_copy(sb[:], ps[:])
                    kv_sb.append(sb)
                    sq = tp.tile([128, SL], dt, name=f"sqk{o}", tag="sq", bufs=3)
                    nc.scalar.activation(sq[:], ps[:], AF.Square)
                    nc.tensor.matmul(ssq_kv[:], ocol[:], sq[:], start=(o == 0), stop=(o == KCH - 1))

                # k_pe rope from host tables (transposed layout)
                krot = tp.tile([ROPE, SL], f32, name="krot", tag="krot", bufs=1)
                nc.vector.tensor_scalar(out=krot[0:32, :], in0=ps_pe[32:64, :], scalar1=-1.0, scalar2=None, op0=ALU.mult)
                nc.vector.tensor_copy(krot[32:64, :], ps_pe[0:32, :])
                kro = tp.tile([ROPE, SL], f32, name="kro", tag="kro", bufs=1)
                nc.vector.tensor_mul(kro[:], ps_pe[:], cosl_t[:])
                krs = tp.tile([ROPE, SL], f32, name="krs", tag="krs", bufs=1)
                nc.vector.tensor_mul(krs[:], krot[:], sinl_t[:])
                kfin = tp.tile([ROPE, SL], dt, name="kfin", tag="kfin", bufs=1)
                nc.vector.tensor_add(kfin[:], kro[:], krs[:])
                nc.scalar.dma_start(out=ag_in_kv[KVLR:KVLR + ROPE, :], in_=kfin[:])

                # kv rms scale + store
                ms_kv = tp.tile([1, SL], f32, name="ms_kv", tag="ms", bufs=2)
                nc.scalar.activation(ms_kv[:], ssq_kv[:], AF.Sqrt, scale=1.0 / KVLR)
                rkv = tp.tile([1, SL], f32, name="rkv", tag="rr", bufs=2)
                nc.vector.reciprocal_approx_fast(out=rkv[:], in_=ms_kv[:])
                rkv_r = tp.tile([1, SL], f32r, name="rkv_r", tag="rrr", bufs=2)
                with nc.allow_low_precision(reason="f32r rounding of rms scale"):
                    nc.vector.tensor_copy(rkv_r[:], rkv[:])
                bc_kv = pp1.tile([128, SL], f32, name="bc_kv", tag="bc", bufs=2)
                nc.tensor.matmul(bc_kv[:], orow[:], rkv_r[:], start=True, stop=True)
                for o in range(KCH):
                    sc = tp.tile([128, SL], dt, name=f"sck{o}", tag="sc", bufs=3)
                    nc.vector.tensor_mul(sc[:], kv_sb[o][:], bc_kv[:])
                    nc.scalar.dma_start(out=ag_in_kv[128 * o:128 * (o + 1), :], in_=sc[:])

                nc.gpsimd.collective_compute(
                    "AllGather", ALU.bypass,
                    replica_groups=[list(range(NC_))],
                    ins=[ag_in_kv[:]], outs=[ag_out_kv[:]],
                )

                # ---- q chunks (overlap the kv AllGather) ----
                ssq_q = pp1.tile([1, SL], f32, name="ssq_q")
                qa_sb = []
                for o in range(QCH):
                    ps = a_chunk(KVW + 128 * o, 128, f"q{o}")
                    sb = rp.tile([128, SL], f32, name=f"qa_sb{o}")
                    nc.vector.tensor_copy(sb[:], ps[:])
                    qa_sb.append(sb)
                    sq = tp.tile([128, SL], dt, name=f"sqq{o}", tag="sq", bufs=3)
                    nc.scalar.activation(sq[:], ps[:], AF.Square)
                    nc.tensor.matmul(ssq_q[:], ocol[:], sq[:], start=(o == 0), stop=(o == QCH - 1))
                ms_q = tp.tile([1, SL], f32, name="ms_q", tag="ms", bufs=2)
                nc.scalar.activation(ms_q[:], ssq_q[:], AF.Sqrt, scale=1.0 / QLR)
                rq = tp.tile([1, SL], f32, name="rq", tag="rr", bufs=2)
                nc.vector.reciprocal_approx_fast(out=rq[:], in_=ms_q[:])
                rq_r = tp.tile([1, SL], f32r, name="rq_r", tag="rrr", bufs=2)
                with nc.allow_low_precision(reason="f32r rounding of rms scale"):
                    nc.vector.tensor_copy(rq_r[:], rq[:])
                bc_q = pp1.tile([128, SL], f32, name="bc_q", tag="bc", bufs=2)
                nc.tensor.matmul(bc_q[:], orow[:], rq_r[:], start=True, stop=True)
                for o in range(QCH):
                    sc = tp.tile([128, SL], dt, name=f"scq{o}", tag="sc", bufs=3)
                    nc.vector.tensor_mul(sc[:], qa_sb[o][:], bc_q[:])
                    nc.scalar.dma_start(out=ag_in_q[128 * o:128 * (o + 1), :], in_=sc[:])

                nc.gpsimd.collective_compute(
                    "AllGather", ALU.bypass,
                    replica_groups=[list(range(NC_))],
                    ins=[ag_in_q[:]], outs=[ag_out_q[:]],
                )

            # ================= Stage B: head-parallel attention =================
            with tc.tile_pool(name="sb_res", bufs=1) as rp, \
                 tc.tile_pool(name="sb_tmp", bufs=2) as tp, \
                 tc.tile_pool(name="sb_pt", bufs=4) as ptp, \
                 tc.tile_pool(name="sb_wo", bufs=1) as wsp, \
                 tc.tile_pool(name="sb_ag", bufs=1) as agp, \
                 tc.tile_pool(name="sb_ps", bufs=2, space="PSUM") as pp, \
                 tc.tile_pool(name="sb_ps1", bufs=1, space="PSUM") as pp1:

                # gathered kv reads: kpe + first half on the gpsimd ring
                # (after both AG triggers, so their AGkv wait blocks nothing),
                # second half on the scalar ring (after all stage-A scalar work)
                ckvp = tc.alloc_tile_pool(name="sb_ckv", bufs=1)
                ckv_g = [ckvp.tile([128, S], dt, name=f"ckv_g{j}") for j in range(KCH)]
                for r in range(NC_):
                    nc.gpsimd.dma_start(out=kpe_g[:, SL * r:SL * (r + 1)],
                                        in_=agkv_r[r, KVLR:KVLR + ROPE, :])
                for j in range(KCH):
                    eng = nc.gpsimd if j < 2 else nc.scalar
                    for r in range(NC_):
                        eng.dma_start(out=ckv_g[j][:, SL * r:SL * (r + 1)],
                                      in_=agkv_r[r, 128 * j:128 * (j + 1), :])

                # K^T and V (both heads)
                for h in range(HPC):
                    for kc in range(S // 512):
                        ps = pp.tile([128, 512], f32, name=f"kt_ps{h}_{kc}", tag="mm_ps", bufs=2)
                        for l in range(KCH):
                            nc.tensor.matmul(ps[:], wkk_t[l][:, NOPE * h:NOPE * (h + 1)],
                                             ckv_g[l][:, 512 * kc:512 * (kc + 1)],
                                             start=(l == 0), stop=(l == KCH - 1))
                        nc.vector.tensor_copy(kT[h][:, 512 * kc:512 * (kc + 1)], ps[:])
                for kb in range(NKB):
                    ps = pp.tile([128, HPC * VD], f32, name=f"v_ps{kb}", tag="mm_ps", bufs=2)
                    for l in range(KCH):
                        nc.tensor.matmul(ps[:], ckv_g[l][:, 128 * kb:128 * (kb + 1)], wkv_t[l][:],
                                         start=(l == 0), stop=(l == KCH - 1))
                    nc.vector.tensor_copy(v_t[kb][:], ps[:])
                ckvp.release()
                qap = tc.alloc_tile_pool(name="sb_qa", bufs=2)

                # Wo preload: 16 full-row tiles (4KB lines, 8x fewer descriptors
                # than column tiles); issued after the stage-A sync stream
                wo_t = []
                for c in range(HCH):
                    t = wsp.tile([128, HID], dt, name=f"wo_t{c}", tag="wo_t", bufs=16)
                    nc.sync.dma_start(out=t[:], in_=wo[128 * c:128 * (c + 1), :])
                    wo_t.append(t)

                def wo_map(c, col):
                    return wo_t[c][:, 512 * col:512 * (col + 1)]

                # ---- q projections for BOTH heads (gathered q_a read once),
                # then per head: attention + AllToAll ----
                att_gh = [agp.tile([128, NC_ * SL], dt, name=f"att_gh{h}") for h in range(HPC)]
                wo1_sb = {}
                qn_sb = {}
                qp_sb = {}
                for h in range(HPC):
                    if h == 0:
                        for p in range(NPANEL):
                            qs = slice(PANEL * p, PANEL * (p + 1))
                            qa_p = []
                            for l in range(QCH):
                                t = qap.tile([128, PANEL], dt, name=f"qa_p{p}_{l}", tag=f"qa_p{l}", bufs=2)
                                eng = nc.scalar if l % 2 == 0 else nc.sync
                                for r in range(2):
                                    eng.dma_start(out=t[:, SL * r:SL * (r + 1)],
                                                  in_=agq_r[2 * p + r, 128 * l:128 * (l + 1), :])
                                qa_p.append(t)
                            for hh in range(HPC):
                                hcol = 256 * hh
                                ps_qn = pp.tile([128, PANEL], f32, name=f"qn_ps{hh}_{p}", tag="mm_ps", bufs=2)
                                for l in range(QCH):
                                    nc.tensor.matmul(ps_qn[:], wqb_t[l][:, hcol:hcol + NOPE], qa_p[l][:],
                                                     start=(l == 0), stop=(l == QCH - 1))
                                ps_qr = pp.tile([128, PANEL], f32, name=f"qr_ps{hh}_{p}", tag="mm_ps", bufs=2)
                                for l in range(QCH):
                                    nc.tensor.matmul(ps_qr[:], wqb_t[l][:, hcol + NOPE:hcol + 256], qa_p[l][:],
                                                     start=(l == 0), stop=(l == QCH - 1))
                                qn = rp.tile([128, PANEL], dt, name=f"qn_sb{hh}_{p}", tag=f"qn{hh}{p}", bufs=1)
                                nc.vector.tensor_copy(qn[:], ps_qn[:])
                                qn_sb[(hh, p)] = qn
                                qt1 = tp.tile([ROPE, PANEL], f32, name=f"qt1_{hh}_{p}", tag="qt1", bufs=1)
                                nc.vector.tensor_mul(qt1[:], ps_qr[0:ROPE, :], cosa_t[:, qs])
                                qt2 = tp.tile([ROPE, PANEL], f32, name=f"qt2_{hh}_{p}", tag="qt2", bufs=1)
                                nc.vector.tensor_mul(qt2[:], ps_qr[ROPE:2 * ROPE, :], sina_t[:, qs])
                                qp = rp.tile([ROPE, PANEL], dt, name=f"qp_sb{hh}_{p}", tag=f"qp{hh}{p}", bufs=1)
                                nc.vector.tensor_add(qp[:], qt1[:], qt2[:])
                                qp_sb[(hh, p)] = qp

                    for p in range(NPANEL):
                        if h == 1:
                            # Wo pass 1: the h0-half of the output projection,
                            # hidden under attention-h1 (needs only att_gh[0])
                            for t in (2 * p, 2 * p + 1):
                                col, qb = t // 2, t % 2
                                ps_w = pp.tile([128, 512], f32, name=f"o1_ps{t}", tag="mm_ps", bufs=2)
                                for j in range(NC_):
                                    nc.tensor.matmul(
                                        ps_w[:],
                                        att_gh[0][:, j * SL + 128 * qb:j * SL + 128 * (qb + 1)],
                                        wo_map(2 * j, col),
                                        start=(j == 0), stop=(j == NC_ - 1))
                                w1 = rp.tile([128, 512], dt, name=f"wo1_sb{t}", tag=f"wo1_{t}", bufs=1)
                                nc.vector.tensor_copy(w1[:], ps_w[:])
                                wo1_sb[t] = w1
                        nkb = 4 * (p + 1)
                        ps_at = pp1.tile([128, PANEL], f32, name=f"at_ps{h}_{p}", tag="at_ps", bufs=2)
                        acc = tp.tile([128, PANEL], f32, name=f"acc{h}_{p}", tag="acc", bufs=2)
                        pts = {}

                        for kb in range(nkb):
                            ps_sc = pp.tile([128, PANEL], f32, name=f"sc_ps{h}_{p}_{kb}", tag="sc_ps", bufs=3)
                            nc.tensor.matmul(ps_sc[:], kT[h][:, 128 * kb:128 * (kb + 1)], qn_sb[(h, p)][:],
                                             start=True, stop=False)
                            nc.tensor.matmul(ps_sc[:], kpe_g[:, 128 * kb:128 * (kb + 1)], qp_sb[(h, p)][:],
                                             start=False, stop=True)
                            pt = ptp.tile([128, PANEL], dt, name=f"pt{h}_{p}_{kb}", tag="pt", bufs=4)
                            nc.scalar.activation(pt[:], ps_sc[:], AF.Exp, scale=SM_SCALE)
                            if kb >= 4 * p:
                                j = kb - 4 * p
                                nc.gpsimd.affine_select(
                                    out=pt[:], in_=pt[:],
                                    pattern=[[1, PANEL]],
                                    compare_op=ALU.is_ge,
                                    fill=0.0,
                                    base=-128 * j,
                                    channel_multiplier=-1)
                            pts[kb] = pt
                            # prob-sum accumulated on the vector engine
                            if kb == 0:
                                nc.vector.tensor_copy(acc[:], pt[:])
                            else:
                                nc.vector.tensor_add(acc[:], acc[:], pt[:])
                            nc.tensor.matmul(ps_at[:], v_t[kb][:, VD * h:VD * (h + 1)], pts[kb][:],
                                             start=(kb == 0), stop=(kb == nkb - 1))
                        ps_sum = pp1.tile([1, PANEL], f32, name=f"sum_ps{h}_{p}", tag="sm_bc", bufs=1)
                        nc.tensor.matmul(ps_sum[:], ocol32[:], acc[:], start=True, stop=True)
                        rec = tp.tile([1, PANEL], f32, name=f"rec{h}_{p}", tag="rec", bufs=2)
                        nc.vector.reciprocal_approx_fast(out=rec[:], in_=ps_sum[:])
                        rec_r = tp.tile([1, PANEL], f32r, name=f"rec_r{h}_{p}", tag="rec_r", bufs=2)
                        with nc.allow_low_precision(reason="f32r rounding of softmax recip"):
                            nc.vector.tensor_copy(rec_r[:], rec[:])
                        bc = pp1.tile([128, PANEL], f32, name=f"bc_ps{h}_{p}", tag="sm_bc", bufs=1)
                        nc.tensor.matmul(bc[:], orow[:], rec_r[:], start=True, stop=True)
                        bc_sb = tp.tile([128, PANEL], f32, name=f"bc_sb{h}_{p}", tag="bc_sb", bufs=2)
                        nc.vector.tensor_copy(bc_sb[:], bc[:])
                        at_p = tp.tile([128, PANEL], dt, name=f"at_p{h}_{p}", tag="at_p", bufs=2)
                        nc.vector.tensor_mul(at_p[:], ps_at[:], bc_sb[:])
                        for r in range(2):
                            j = 2 * p + r
                            nc.scalar.dma_start(
                                out=a2a_in[h][j * VD:(j + 1) * VD, :],
                                in_=at_p[:, SL * r:SL * (r + 1)])
                    nc.gpsimd.collective_compute(
                        "AllToAll", ALU.bypass,
                        replica_groups=[list(range(NC_))],
                        ins=[a2a_in[h][:]], outs=[a2a_out[h][:]],
                    )
                    # gathered attn reads on gpsimd: the wait on the AllToAll
                    # must not block the sync/scalar queues
                    for j in range(NC_):
                        nc.gpsimd.dma_start(out=att_gh[h][:, SL * j:SL * (j + 1)],
                                            in_=a2a_out[h][128 * j:128 * (j + 1), :])

                # ---- Wo pass 2: h1-half + combine ----
                for t in range(8):
                    col, qb = t // 2, t % 2
                    ps = pp.tile([128, 512], f32, name=f"o_ps{t}", tag="mm_ps", bufs=2)
                    for j in range(NC_):
                        nc.tensor.matmul(
                            ps[:],
                            att_gh[1][:, j * SL + 128 * qb:j * SL + 128 * (qb + 1)],
                            wo_map(2 * j + 1, col),
                            start=(j == 0), stop=(j == NC_ - 1))
                    osb = tp.tile([128, 512], f32, name=f"osb{t}", tag="osb", bufs=2)
                    nc.vector.tensor_add(osb[:], ps[:], wo1_sb[t][:])
                    nc.sync.dma_start(out=out_loc[128 * qb:128 * (qb + 1), 512 * col:512 * (col + 1)], in_=osb[:])
                qap.release()

    nc.compile()
    return nc


def _to_dt(a, dt):
    if dt == bf16:
        return np.ascontiguousarray(a.astype(ml_dtypes.bfloat16))
    return np.ascontiguousarray(a.astype(np.float32))


def _prepare_inputs(dt, hidden_states, position_ids, Wqa, qa_ln_w, Wqb, Wkva, kv_ln_w, Wkvb, Wo):
    perm = np.concatenate([np.arange(0, ROPE, 2), np.arange(1, ROPE, 2)])
    X = np.asarray(hidden_states, np.float32).reshape(S, HID)
    Wqa = np.asarray(Wqa, np.float32)
    Wkva = np.asarray(Wkva, np.float32)
    # kv cols first so stage A can load + compute the kv path before q
    wa = np.concatenate([Wkva[:, :KVLR], Wkva[:, KVLR:][:, perm], Wqa], axis=1)  # (2048, 2112)
    wqb_base = np.asarray(Wqb, np.float32) * np.asarray(qa_ln_w, np.float32)[:, None]
    wkvb_base = np.asarray(Wkvb, np.float32) * np.asarray(kv_ln_w, np.float32)[:, None]
    Wo = np.asarray(Wo, np.float32)

    head_blocks = []
    for h in range(NH):
        cols = wqb_base[:, 192 * h:192 * (h + 1)]
        nope = cols[:, :NOPE]
        pe_d = cols[:, NOPE:][:, perm]
        rot = np.concatenate([-pe_d[:, 32:], pe_d[:, :32]], axis=1)
        head_blocks.append(np.concatenate([nope, pe_d, rot], axis=1))  # (1536, 256)
    k_blocks = [wkvb_base[:, 256 * h:256 * h + NOPE] for h in range(NH)]
    v_blocks = [wkvb_base[:, 256 * h + NOPE:256 * (h + 1)] for h in range(NH)]

    # host-precomputed RoPE tables in deinterleaved layout: row d uses
    # inv_freq[d % 32], column t is position t
    pos = np.asarray(position_ids, np.float32).reshape(S)
    inv = (1.0 / (THETA ** (np.arange(0, ROPE, 2, dtype=np.float32) / ROPE))).astype(np.float32)
    invf = np.concatenate([inv, inv])                      # (64,)
    emb = invf[:, None] * pos[None, :]                     # (64, S)
    cos_np = np.cos(emb).astype(np.float32)
    sin_np = np.sin(emb).astype(np.float32)

    wo_d = _to_dt(Wo, dt)
    ones_col_d = _to_dt(np.ones((128, 1), np.float32), dt)
    cos_all_d = _to_dt(cos_np, dt)
    sin_all_d = _to_dt(sin_np, dt)

    in_maps = []
    for c in range(NC_):
        rows = slice(SL * c, SL * (c + 1))
        in_maps.append({
            "xwa": _to_dt(np.concatenate([X[rows, :].T, wa], axis=1), dt),
            "wqb": _to_dt(np.concatenate([head_blocks[HPC * c + h] for h in range(HPC)], axis=1), dt),
            "wkvb_k": _to_dt(np.concatenate([k_blocks[HPC * c + h] for h in range(HPC)], axis=1), dt),
            "wkvb_v": _to_dt(np.concatenate([v_blocks[HPC * c + h] for h in range(HPC)], axis=1), dt),
            "wo": wo_d,
            "ones_col": ones_col_d,
            "ones_col32": np.ones((128, 1), np.float32),
            "ones_row": np.ones((1, 128), np.float32),
            "cos_loc": np.ascontiguousarray(cos_np[:, rows]),
            "sin_loc": np.ascontiguousarray(sin_np[:, rows]),
            "cos_all": cos_all_d,
            "sin_all": sin_all_d,
        })
    return in_maps


def run(inputs, trace=False, trace_cores=None, dt=None):
    dt = dt if dt is not None else DT
    key = ("nc", str(dt))
    if key not in _CACHE:
        _CACHE[key] = build_program(dt)
    nc = _CACHE[key]
    in_maps = _prepare_inputs(dt, **inputs)
    res = run_bass_kernel_spmd(nc, in_maps, list(range(NC_)), trace=trace,
                               trace_cores=trace_cores)
    out = np.concatenate([res.results[c]["out_loc"] for c in range(NC_)], axis=0)
    return out.reshape(1, S, HID), res


def kernel(**inputs) -> np.ndarray:
    out, _ = run(inputs, trace=False)
    return out


# revision 44
# speedup vs baseline: 1.0589x; 1.0230x over previous
"""DeepseekV3 MLA flash-attention prefill kernel for 8 Trainium2 NeuronCores.

Sharding strategy (SPMD, one program for all 8 cores):
  Stage A (sequence-parallel, feature-major): core c computes the low-rank
    down-projections q_a = rms_norm(X @ Wqa), c_kv = rms_norm(ckv[:, :512]),
    k_pe(roped) for its 256 rows directly in transposed layout
    (lhsT = weight chunks, rhs = X^T).  The kv path goes first so its
    AllGather fires early and overlaps the q-chunk matmuls; the q AllGather
    then overlaps the K^T / V up-projections.  RoPE sin/cos tables are
    precomputed on the host (no on-device Sin/range-reduction).
  Stage B (head-parallel): core c owns heads {2c, 2c+1}: per head, q
    projections (Wqb + RoPE via duplicated rot columns) are computed per
    panel, then causal attention runs in (k, q) layout: softmax without
    max-subtraction, fully-masked k-blocks skipped, diagonal blocks masked
    with GpSimd affine_select.  The prob-sum is accumulated on the vector
    engine (PE only does one ones-matmul per panel), and the per-q
    normalization is folded into the attn^T eviction.  Each head's attn^T
    is exchanged with its own AllToAll so the first overlaps the second
    head.  SBUF for the K/V/ckv tiles is reserved ahead of stage A so their
    DMAs don't wait on stage-A anti-dependencies.
  Each core then computes its 256 output rows against the full Wo.
  Host concatenates.
"""

import sys

if '/opt/trn_rl_repo' not in sys.path:
    sys.path.insert(0, '/opt/trn_rl_repo')

import numpy as np
import ml_dtypes

import concourse.bass as bass
import concourse.mybir as mybir
import concourse.tile as tile
from concourse import bacc
from concourse.bass_utils import run_bass_kernel_spmd

f32 = mybir.dt.float32
f32r = mybir.dt.float32r
bf16 = mybir.dt.bfloat16
i32 = mybir.dt.int32
AF = mybir.ActivationFunctionType
ALU = mybir.AluOpType

NC_ = 8            # cores
S = 2048           # sequence
HID = 2048
QLR = 1536         # q lora rank
KVLR = 512         # kv lora rank
ROPE = 64
NOPE = 128
VD = 128
NH = 16
HPC = NH // NC_    # heads per core = 2
SL = S // NC_      # rows per core = 256
PANEL = 512        # q panel width
NPANEL = S // PANEL
NKB = S // 128     # 16 k blocks
QCH = QLR // 128   # 12
KCH = KVLR // 128  # 4
HCH = HID // 128   # 16
KVW = KVLR + ROPE  # 576 = kv + rope cols of wa
THETA = 10000.0
SM_SCALE = float((NOPE + ROPE) ** -0.5)

DT = bf16          # matmul dtype

_CACHE = {}


def build_program(dt):
    nc = bacc.Bacc("TRN2", target_bir_lowering=False, debug=False, num_devices=NC_)

    def din(name, shape):
        return nc.dram_tensor(name, shape, dt, kind="ExternalInput")

    # ---- external I/O (per-core data) ----
    # [X^T | Wkva(kv) | Wkva(pe, deint) | Wqa] merged so each 128-row chunk
    # loads with one fat DMA (4.7KB per partition line)
    XWAW = SL + KVW + QLR
    xwa = din("xwa", [HID, XWAW])
    wqb = din("wqb", [QLR, HPC * 256])          # [nope|pe_d|rot] per head
    wkvb_k = din("wkvb_k", [KVLR, HPC * NOPE])
    wkvb_v = din("wkvb_v", [KVLR, HPC * VD])
    wo = din("wo", [NH * VD, HID])
    ones_col = din("ones_col", [128, 1])
    ones_col32 = nc.dram_tensor("ones_col32", [128, 1], f32, kind="ExternalInput")
    ones_row = nc.dram_tensor("ones_row", [1, 128], f32, kind="ExternalInput")
    cos_loc = nc.dram_tensor("cos_loc", [ROPE, SL], f32, kind="ExternalInput")
    sin_loc = nc.dram_tensor("sin_loc", [ROPE, SL], f32, kind="ExternalInput")
    cos_all = din("cos_all", [ROPE, S])
    sin_all = din("sin_all", [ROPE, S])
    out_loc = nc.dram_tensor("out_loc", [SL, HID], f32, kind="ExternalOutput")

    AGKV_R = KVLR + ROPE   # 576 rows in the kv AllGather

    with tile.TileContext(nc) as tc:
        with tc.tile_pool(name="dram", bufs=1, space="DRAM") as dpool, \
             tc.tile_pool(name="consts", bufs=1) as cpool, \
             tc.tile_pool(name="sb_w", bufs=1) as wbp, \
             tc.tile_pool(name="sb_kv", bufs=1) as kvp:
            dummy_in = dpool.tile([128, 1], dt)
            dummy_out = dpool.tile([NC_ * 128, 1], dt, addr_space="Shared")
            ag_in_kv = dpool.tile([AGKV_R, SL], dt)
            ag_out_kv = dpool.tile([NC_ * AGKV_R, SL], dt, addr_space="Shared")
            ag_in_q = dpool.tile([QCH * 128, SL], dt)
            ag_out_q = dpool.tile([NC_ * QCH * 128, SL], dt, addr_space="Shared")
            a2a_in = [dpool.tile([NC_ * VD, SL], dt, name=f"a2a_in{h}") for h in range(HPC)]
            a2a_out = [dpool.tile([NC_ * VD, SL], dt, name=f"a2a_out{h}") for h in range(HPC)]

            agkv_r = ag_out_kv.rearrange("(r c) q -> r c q", r=NC_)
            agq_r = ag_out_q.rearrange("(r c) q -> r c q", r=NC_)

            ocol = cpool.tile([128, 1], dt)
            ocol32 = cpool.tile([128, 1], f32)
            orow = cpool.tile([1, 128], f32r)
            cosa_t = cpool.tile([ROPE, S], dt)
            sina_t = cpool.tile([ROPE, S], dt)
            nc.sync.dma_start(out=ocol[:], in_=ones_col[:])
            nc.sync.dma_start(out=ocol32[:], in_=ones_col32[:])
            nc.sync.dma_start(out=orow[:], in_=ones_row[:].bitcast(f32r))

            # tiny warm-up collective: absorbs the ~11us CC-core first-call
            # latency and aligns rank start skew before the real AllGathers
            nc.scalar.dma_start(out=dummy_in[:], in_=ocol[:])
            nc.gpsimd.collective_compute(
                "AllGather", ALU.bypass,
                replica_groups=[list(range(NC_))],
                ins=[dummy_in[:]], outs=[dummy_out[:]],
            )

            # stage-B K/V-side tiles reserved ahead of stage A so their DMAs /
            # writes don't wait on stage-A SBUF anti-dependencies
            kpe_g = kvp.tile([ROPE, S], dt, name="kpe_g")
            kT = [kvp.tile([128, S], dt, name=f"kT{h}") for h in range(HPC)]
            v_t = [kvp.tile([128, HPC * VD], dt, name=f"v_t{kb}") for kb in range(NKB)]

            # ================= Stage A: transposed down projections =================
            with tc.tile_pool(name="sa_x", bufs=1) as xp, \
                 tc.tile_pool(name="sa_res", bufs=1) as rp, \
                 tc.tile_pool(name="sa_tmp", bufs=2) as tp, \
                 tc.tile_pool(name="sa_ps", bufs=3, space="PSUM") as pp, \
                 tc.tile_pool(name="sa_ps1", bufs=1, space="PSUM") as pp1:

                cosl_t = rp.tile([ROPE, SL], f32, name="cosl_t")
                sinl_t = rp.tile([ROPE, SL], f32, name="sinl_t")
                nc.sync.dma_start(out=cosl_t[:], in_=cos_loc[:])
                nc.sync.dma_start(out=sinl_t[:], in_=sin_loc[:])

                # merged x/weight chunk loads; 4 partition-quarter DMAs per
                # chunk so one chunk's bytes drain on 4 engines in parallel
                xwa_t = []
                for k in range(HCH):
                    t = xp.tile([128, XWAW], dt, name=f"xwa_{k}")
                    for q4 in range(4):
                        nc.sync.dma_start(out=t[32 * q4:32 * (q4 + 1), :],
                                          in_=xwa[128 * k + 32 * q4:128 * k + 32 * (q4 + 1), :])
                    xwa_t.append(t)
                xts = [t[:, 0:SL] for t in xwa_t]
                # stage-B up-projection weights next in the sync queue
                wkk_t = []
                wkv_t = []
                for l in range(KCH):
                    t = wbp.tile([128, HPC * NOPE], dt, name=f"wkk_t{l}")
                    nc.sync.dma_start(out=t[:], in_=wkvb_k[128 * l:128 * (l + 1), :])
                    wkk_t.append(t)
                    t2 = wbp.tile([128, HPC * VD], dt, name=f"wkv_t{l}")
                    nc.sync.dma_start(out=t2[:], in_=wkvb_v[128 * l:128 * (l + 1), :])
                    wkv_t.append(t2)
                wqb_t = []
                for l in range(QCH):
                    t = wbp.tile([128, HPC * 256], dt, name=f"wqb_t{l}")
                    nc.sync.dma_start(out=t[:], in_=wqb[128 * l:128 * (l + 1), :])
                    wqb_t.append(t)
                nc.sync.dma_start(out=cosa_t[:], in_=cos_all[:])
                nc.sync.dma_start(out=sina_t[:], in_=sin_all[:])

                def a_chunk(c0, width, tag):
                    ps = pp.tile([width, SL], f32, name=f"ps_{tag}", tag="a_ps", bufs=3)
                    for hc in range(HCH):
                        nc.tensor.matmul(ps[:], xwa_t[hc][:, SL + c0:SL + c0 + width], xts[hc],
                                         start=(hc == 0), stop=(hc == HCH - 1))
                    return ps

                # ---- k_pe + kv chunks first: unblock the kv AllGather ASAP ----
                ps_pe = a_chunk(KVLR, ROPE, "pe")

                ssq_kv = pp1.tile([1, SL], f32, name="ssq_kv")
                kv_sb = []
                sqs = []
                for o in range(KCH):
                    ps = a_chunk(128 * o, 128, f"kv{o}")
                    sb = rp.tile([128, SL], f32, name=f"kv_sb{o}")
                    nc.vector.tensor_copy(sb[:], ps[:])
                    kv_sb.append(sb)
                    sq = tp.tile([128, SL], dt, name=f"sqk{o}", tag=f"sqk{o}", bufs=1)
                    nc.scalar.activation(sq[:], ps[:], AF.Square)
                    sqs.append(sq)
                # ssq matmuls deferred so the tensor queue never waits on the
                # scalar-engine squares mid-stream
                for o in range(KCH):
                    nc.tensor.matmul(ssq_kv[:], ocol[:], sqs[o][:], start=(o == 0), stop=(o == KCH - 1))

                # k_pe rope from host tables (transposed layout)
                krot = tp.tile([ROPE, SL], f32, name="krot", tag="krot", bufs=1)
                nc.vector.tensor_scalar(out=krot[0:32, :], in0=ps_pe[32:64, :], scalar1=-1.0, scalar2=None, op0=ALU.mult)
                nc.vector.tensor_copy(krot[32:64, :], ps_pe[0:32, :])
                kro = tp.tile([ROPE, SL], f32, name="kro", tag="kro", bufs=1)
                nc.vector.tensor_mul(kro[:], ps_pe[:], cosl_t[:])
                krs = tp.tile([ROPE, SL], f32, name="krs", tag="krs", bufs=1)
                nc.vector.tensor_mul(krs[:], krot[:], sinl_t[:])
                kfin = tp.tile([ROPE, SL], dt, name="kfin", tag="kfin", bufs=1)
                nc.vector.tensor_add(kfin[:], kro[:], krs[:])
                nc.scalar.dma_start(out=ag_in_kv[KVLR:KVLR + ROPE, :], in_=kfin[:])

                # kv rms scale + store
                ms_kv = tp.tile([1, SL], f32, name="ms_kv", tag="ms", bufs=2)
                nc.scalar.activation(ms_kv[:], ssq_kv[:], AF.Sqrt, scale=1.0 / KVLR)
                rkv = tp.tile([1, SL], f32, name="rkv", tag="rr", bufs=2)
                nc.vector.reciprocal_approx_fast(out=rkv[:], in_=ms_kv[:])
                rkv_r = tp.tile([1, SL], f32r, name="rkv_r", tag="rrr", bufs=2)
                with nc.allow_low_precision(reason="f32r rounding of rms scale"):
                    nc.vector.tensor_copy(rkv_r[:], rkv[:])
                bc_kv = pp1.tile([128, SL], f32, name="bc_kv", tag="bc", bufs=2)
                nc.tensor.matmul(bc_kv[:], orow[:], rkv_r[:], start=True, stop=True)
                for o in range(KCH):
                    sc = tp.tile([128, SL], dt, name=f"sck{o}", tag="sc", bufs=3)
                    nc.vector.tensor_mul(sc[:], kv_sb[o][:], bc_kv[:])
                    nc.scalar.dma_start(out=ag_in_kv[128 * o:128 * (o + 1), :], in_=sc[:])

                nc.gpsimd.collective_compute(
                    "AllGather", ALU.bypass,
                    replica_groups=[list(range(NC_))],
                    ins=[ag_in_kv[:]], outs=[ag_out_kv[:]],
                )

                # ---- q chunks (overlap the kv AllGather) ----
                ssq_q = pp1.tile([1, SL], f32, name="ssq_q")
                qa_sb = []
                sqs = []
                for o in range(QCH):
                    ps = a_chunk(KVW + 128 * o, 128, f"q{o}")
                    sb = rp.tile([128, SL], f32, name=f"qa_sb{o}")
                    nc.vector.tensor_copy(sb[:], ps[:])
                    qa_sb.append(sb)
                    sq = tp.tile([128, SL], dt, name=f"sqq{o}", tag=f"sqq{o}", bufs=1)
                    nc.scalar.activation(sq[:], ps[:], AF.Square)
                    sqs.append(sq)
                for o in range(QCH):
                    nc.tensor.matmul(ssq_q[:], ocol[:], sqs[o][:], start=(o == 0), stop=(o == QCH - 1))
                ms_q = tp.tile([1, SL], f32, name="ms_q", tag="ms", bufs=2)
                nc.scalar.activation(ms_q[:], ssq_q[:], AF.Sqrt, scale=1.0 / QLR)
                rq = tp.tile([1, SL], f32, name="rq", tag="rr", bufs=2)
                nc.vector.reciprocal_approx_fast(out=rq[:], in_=ms_q[:])
                rq_r = tp.tile([1, SL], f32r, name="rq_r", tag="rrr", bufs=2)
                with nc.allow_low_precision(reason="f32r rounding of rms scale"):
                    nc.vector.tensor_copy(rq_r[:], rq[:])
                bc_q = pp1.tile([128, SL], f32, name="bc_q", tag="bc", bufs=2)
                nc.tensor.matmul(bc_q[:], orow[:], rq_r[:], start=True, stop=True)
                for o in range(QCH):
                    sc = tp.tile([128, SL], dt, name=f"scq{o}", tag="sc", bufs=3)
                    nc.vector.tensor_mul(sc[:], qa_sb[o][:], bc_q[:])
                    nc.scalar.dma_start(out=ag_in_q[128 * o:128 * (o + 1), :], in_=sc[:])

                nc.gpsimd.collective_compute(
                    "AllGather", ALU.bypass,
                    replica_groups=[list(range(NC_))],
                    ins=[ag_in_q[:]], outs=[ag_out_q[:]],
                )

            # ================= Stage B: head-parallel attention =================
            with tc.tile_pool(name="sb_res", bufs=1) as rp, \
                 tc.tile_pool(name="sb_tmp", bufs=2) as tp, \
                 tc.tile_pool(name="sb_pt", bufs=4) as ptp, \
                 tc.tile_pool(name="sb_wo", bufs=1) as wsp, \
                 tc.tile_pool(name="sb_ag", bufs=1) as agp, \
                 tc.tile_pool(name="sb_ps", bufs=2, space="PSUM") as pp, \
                 tc.tile_pool(name="sb_ps1", bufs=1, space="PSUM") as pp1:

                # gathered kv reads: kpe + first half on the gpsimd ring
                # (after both AG triggers, so their AGkv wait blocks nothing),
                # second half on the scalar ring (after all stage-A scalar work)
                ckvp = tc.alloc_tile_pool(name="sb_ckv", bufs=1)
                ckv_g = [ckvp.tile([128, S], dt, name=f"ckv_g{j}") for j in range(KCH)]
                for r in range(NC_):
                    nc.gpsimd.dma_start(out=kpe_g[:, SL * r:SL * (r + 1)],
                                        in_=agkv_r[r, KVLR:KVLR + ROPE, :])
                for j in range(KCH):
                    eng = nc.gpsimd if j < 2 else nc.scalar
                    for r in range(NC_):
                        eng.dma_start(out=ckv_g[j][:, SL * r:SL * (r + 1)],
                                      in_=agkv_r[r, 128 * j:128 * (j + 1), :])

                # K^T and V (both heads)
                for h in range(HPC):
                    for kc in range(S // 512):
                        ps = pp.tile([128, 512], f32, name=f"kt_ps{h}_{kc}", tag="mm_ps", bufs=2)
                        for l in range(KCH):
                            nc.tensor.matmul(ps[:], wkk_t[l][:, NOPE * h:NOPE * (h + 1)],
                                             ckv_g[l][:, 512 * kc:512 * (kc + 1)],
                                             start=(l == 0), stop=(l == KCH - 1))
                        nc.vector.tensor_copy(kT[h][:, 512 * kc:512 * (kc + 1)], ps[:])
                for kb in range(NKB):
                    ps = pp.tile([128, HPC * VD], f32, name=f"v_ps{kb}", tag="mm_ps", bufs=2)
                    for l in range(KCH):
                        nc.tensor.matmul(ps[:], ckv_g[l][:, 128 * kb:128 * (kb + 1)], wkv_t[l][:],
                                         start=(l == 0), stop=(l == KCH - 1))
                    nc.vector.tensor_copy(v_t[kb][:], ps[:])
                ckvp.release()
                qap = tc.alloc_tile_pool(name="sb_qa", bufs=2)

                # Wo: 16 full-row tiles (4KB lines); DMAs are issued after the
                # h0 AllToAll so they don't steal HBM bandwidth from the
                # critical q AllGather mesh
                wo_t = [wsp.tile([128, HID], dt, name=f"wo_t{c}", tag="wo_t", bufs=16)
                        for c in range(HCH)]

                def wo_map(c, col):
                    return wo_t[c][:, 512 * col:512 * (col + 1)]

                # ---- q projections for BOTH heads (gathered q_a read once),
                # then per head: attention + AllToAll ----
                att_gh = [agp.tile([128, NC_ * SL], dt, name=f"att_gh{h}") for h in range(HPC)]
                wo1_sb = {}
                qn_sb = {}
                qp_sb = {}
                for h in range(HPC):
                    if h == 0:
                        for p in range(NPANEL):
                            qs = slice(PANEL * p, PANEL * (p + 1))
                            qa_p = []
                            for l in range(QCH):
                                t = qap.tile([128, PANEL], dt, name=f"qa_p{p}_{l}", tag=f"qa_p{l}", bufs=2)
                                eng = nc.scalar if l % 2 == 0 else nc.sync
                                for r in range(2):
                                    eng.dma_start(out=t[:, SL * r:SL * (r + 1)],
                                                  in_=agq_r[2 * p + r, 128 * l:128 * (l + 1), :])
                                qa_p.append(t)
                            for hh in range(HPC):
                                hcol = 256 * hh
                                ps_qn = pp.tile([128, PANEL], f32, name=f"qn_ps{hh}_{p}", tag="mm_ps", bufs=2)
                                for l in range(QCH):
                                    nc.tensor.matmul(ps_qn[:], wqb_t[l][:, hcol:hcol + NOPE], qa_p[l][:],
                                                     start=(l == 0), stop=(l == QCH - 1))
                                ps_qr = pp.tile([128, PANEL], f32, name=f"qr_ps{hh}_{p}", tag="mm_ps", bufs=2)
                                for l in range(QCH):
                                    nc.tensor.matmul(ps_qr[:], wqb_t[l][:, hcol + NOPE:hcol + 256], qa_p[l][:],
                                                     start=(l == 0), stop=(l == QCH - 1))
                                qn = rp.tile([128, PANEL], dt, name=f"qn_sb{hh}_{p}", tag=f"qn{hh}{p}", bufs=1)
                                nc.vector.tensor_copy(qn[:], ps_qn[:])
                                qn_sb[(hh, p)] = qn
                                qt1 = tp.tile([ROPE, PANEL], f32, name=f"qt1_{hh}_{p}", tag="qt1", bufs=1)
                                nc.vector.tensor_mul(qt1[:], ps_qr[0:ROPE, :], cosa_t[:, qs])
                                qt2 = tp.tile([ROPE, PANEL], f32, name=f"qt2_{hh}_{p}", tag="qt2", bufs=1)
                                nc.vector.tensor_mul(qt2[:], ps_qr[ROPE:2 * ROPE, :], sina_t[:, qs])
                                qp = rp.tile([ROPE, PANEL], dt, name=f"qp_sb{hh}_{p}", tag=f"qp{hh}{p}", bufs=1)
                                nc.vector.tensor_add(qp[:], qt1[:], qt2[:])
                                qp_sb[(hh, p)] = qp

                    for p in range(NPANEL):
                        if h == 1:
                            # Wo pass 1: the h0-half of the output projection,
                            # hidden under attention-h1 (needs only att_gh[0])
                            for t in (2 * p, 2 * p + 1):
                                col, qb = t // 2, t % 2
                                ps_w = pp.tile([128, 512], f32, name=f"o1_ps{t}", tag="mm_ps", bufs=2)
                                for j in range(NC_):
                                    nc.tensor.matmul(
                                        ps_w[:],
                                        att_gh[0][:, j * SL + 128 * qb:j * SL + 128 * (qb + 1)],
                                        wo_map(2 * j, col),
                                        start=(j == 0), stop=(j == NC_ - 1))
                                w1 = rp.tile([128, 512], dt, name=f"wo1_sb{t}", tag=f"wo1_{t}", bufs=1)
                                nc.vector.tensor_copy(w1[:], ps_w[:])
                                wo1_sb[t] = w1
                        nkb = 4 * (p + 1)
                        ps_at = pp1.tile([128, PANEL], f32, name=f"at_ps{h}_{p}", tag="at_ps", bufs=2)
                        acc = tp.tile([128, PANEL], f32, name=f"acc{h}_{p}", tag="acc", bufs=2)
                        pts = {}

                        for kb in range(nkb):
                            ps_sc = pp.tile([128, PANEL], f32, name=f"sc_ps{h}_{p}_{kb}", tag="sc_ps", bufs=3)
                            nc.tensor.matmul(ps_sc[:], kT[h][:, 128 * kb:128 * (kb + 1)], qn_sb[(h, p)][:],
                                             start=True, stop=False)
                            nc.tensor.matmul(ps_sc[:], kpe_g[:, 128 * kb:128 * (kb + 1)], qp_sb[(h, p)][:],
                                             start=False, stop=True)
                            pt = ptp.tile([128, PANEL], dt, name=f"pt{h}_{p}_{kb}", tag="pt", bufs=4)
                            nc.scalar.activation(pt[:], ps_sc[:], AF.Exp, scale=SM_SCALE)
                            if kb >= 4 * p:
                                j = kb - 4 * p
                                nc.gpsimd.affine_select(
                                    out=pt[:], in_=pt[:],
                                    pattern=[[1, PANEL]],
                                    compare_op=ALU.is_ge,
                                    fill=0.0,
                                    base=-128 * j,
                                    channel_multiplier=-1)
                            pts[kb] = pt
                            # prob-sum accumulated on the vector engine
                            if kb == 0:
                                nc.vector.tensor_copy(acc[:], pt[:])
                            else:
                                nc.vector.tensor_add(acc[:], acc[:], pt[:])
                            nc.tensor.matmul(ps_at[:], v_t[kb][:, VD * h:VD * (h + 1)], pts[kb][:],
                                             start=(kb == 0), stop=(kb == nkb - 1))
                        ps_sum = pp1.tile([1, PANEL], f32, name=f"sum_ps{h}_{p}", tag="sm_bc", bufs=1)
                        nc.tensor.matmul(ps_sum[:], ocol32[:], acc[:], start=True, stop=True)
                        rec = tp.tile([1, PANEL], f32, name=f"rec{h}_{p}", tag="rec", bufs=2)
                        nc.vector.reciprocal_approx_fast(out=rec[:], in_=ps_sum[:])
                        rec_r = tp.tile([1, PANEL], f32r, name=f"rec_r{h}_{p}", tag="rec_r", bufs=2)
                        with nc.allow_low_precision(reason="f32r rounding of softmax recip"):
                            nc.vector.tensor_copy(rec_r[:], rec[:])
                        bc = pp1.tile([128, PANEL], f32, name=f"bc_ps{h}_{p}", tag="sm_bc", bufs=1)
                        nc.tensor.matmul(bc[:], orow[:], rec_r[:], start=True, stop=True)
                        bc_sb = tp.tile([128, PANEL], f32, name=f"bc_sb{h}_{p}", tag="bc_sb", bufs=2)
                        nc.vector.tensor_copy(bc_sb[:], bc[:])
                        at_p = tp.tile([128, PANEL], dt, name=f"at_p{h}_{p}", tag="at_p", bufs=2)
                        nc.vector.tensor_mul(at_p[:], ps_at[:], bc_sb[:])
                        for r in range(2):
                            j = 2 * p + r
                            nc.scalar.dma_start(
                                out=a2a_in[h][j * VD:(j + 1) * VD, :],
                                in_=at_p[:, SL * r:SL * (r + 1)])
                    nc.gpsimd.collective_compute(
                        "AllToAll", ALU.bypass,
                        replica_groups=[list(range(NC_))],
                        ins=[a2a_in[h][:]], outs=[a2a_out[h][:]],
                    )
                    # gathered attn reads on gpsimd: the wait on the AllToAll
                    # must not block the sync/scalar queues
                    for j in range(NC_):
                        nc.gpsimd.dma_start(out=att_gh[h][:, SL * j:SL * (j + 1)],
                                            in_=a2a_out[h][128 * j:128 * (j + 1), :])
                    if h == 0:
                        for c in range(HCH):
                            nc.sync.dma_start(out=wo_t[c][:], in_=wo[128 * c:128 * (c + 1), :])

                # ---- Wo pass 2: h1-half + combine ----
                for t in range(8):
                    col, qb = t // 2, t % 2
                    ps = pp.tile([128, 512], f32, name=f"o_ps{t}", tag="mm_ps", bufs=2)
                    for j in range(NC_):
                        nc.tensor.matmul(
                            ps[:],
                            att_gh[1][:, j * SL + 128 * qb:j * SL + 128 * (qb + 1)],
                            wo_map(2 * j + 1, col),
                            start=(j == 0), stop=(j == NC_ - 1))
                    osb = tp.tile([128, 512], f32, name=f"osb{t}", tag="osb", bufs=2)
                    nc.vector.tensor_add(osb[:], ps[:], wo1_sb[t][:])
                    for q4 in range(2):
                        nc.sync.dma_start(
                            out=out_loc[128 * qb + 64 * q4:128 * qb + 64 * (q4 + 1),
                                        512 * col:512 * (col + 1)],
                            in_=osb[64 * q4:64 * (q4 + 1), :])
                qap.release()

    nc.compile()
    return nc


def _to_dt(a, dt):
    if dt == bf16:
        return np.ascontiguousarray(a.astype(ml_dtypes.bfloat16))
    return np.ascontiguousarray(a.astype(np.float32))


def _prepare_inputs(dt, hidden_states, position_ids, Wqa, qa_ln_w, Wqb, Wkva, kv_ln_w, Wkvb, Wo):
    perm = np.concatenate([np.arange(0, ROPE, 2), np.arange(1, ROPE, 2)])
    X = np.asarray(hidden_states, np.float32).reshape(S, HID)
    Wqa = np.asarray(Wqa, np.float32)
    Wkva = np.asarray(Wkva, np.float32)
    # kv cols first so stage A can load + compute the kv path before q
    wa = np.concatenate([Wkva[:, :KVLR], Wkva[:, KVLR:][:, perm], Wqa], axis=1)  # (2048, 2112)
    wqb_base = np.asarray(Wqb, np.float32) * np.asarray(qa_ln_w, np.float32)[:, None]
    wkvb_base = np.asarray(Wkvb, np.float32) * np.asarray(kv_ln_w, np.float32)[:, None]
    Wo = np.asarray(Wo, np.float32)

    head_blocks = []
    for h in range(NH):
        cols = wqb_base[:, 192 * h:192 * (h + 1)]
        nope = cols[:, :NOPE]
        pe_d = cols[:, NOPE:][:, perm]
        rot = np.concatenate([-pe_d[:, 32:], pe_d[:, :32]], axis=1)
        head_blocks.append(np.concatenate([nope, pe_d, rot], axis=1))  # (1536, 256)
    k_blocks = [wkvb_base[:, 256 * h:256 * h + NOPE] for h in range(NH)]
    v_blocks = [wkvb_base[:, 256 * h + NOPE:256 * (h + 1)] for h in range(NH)]

    # host-precomputed RoPE tables in deinterleaved layout: row d uses
    # inv_freq[d % 32], column t is position t
    pos = np.asarray(position_ids, np.float32).reshape(S)
    inv = (1.0 / (THETA ** (np.arange(0, ROPE, 2, dtype=np.float32) / ROPE))).astype(np.float32)
    invf = np.concatenate([inv, inv])                      # (64,)
    emb = invf[:, None] * pos[None, :]                     # (64, S)
    cos_np = np.cos(emb).astype(np.float32)
    sin_np = np.sin(emb).astype(np.float32)

    wo_d = _to_dt(Wo, dt)
    ones_col_d = _to_dt(np.ones((128, 1), np.float32), dt)
    cos_all_d = _to_dt(cos_np, dt)
    sin_all_d = _to_dt(sin_np, dt)

    in_maps = []
    for c in range(NC_):
        rows = slice(SL * c, SL * (c + 1))
        in_maps.append({
            "xwa": _to_dt(np.concatenate([X[rows, :].T, wa], axis=1), dt),
            "wqb": _to_dt(np.concatenate([head_blocks[HPC * c + h] for h in range(HPC)], axis=1), dt),
            "wkvb_k": _to_dt(np.concatenate([k_blocks[HPC * c + h] for h in range(HPC)], axis=1), dt),
            "wkvb_v": _to_dt(np.concatenate([v_blocks[HPC * c + h] for h in range(HPC)], axis=1), dt),
            "wo": wo_d,
            "ones_col": ones_col_d,
            "ones_col32": np.ones((128, 1), np.float32),
            "ones_row": np.ones((1, 128), np.float32),
            "cos_loc": np.ascontiguousarray(cos_np[:, rows]),
            "sin_loc": np.ascontiguousarray(sin_np[:, rows]),
            "cos_all": cos_all_d,
            "sin_all": sin_all_d,
        })
    return in_maps


def run(inputs, trace=False, trace_cores=None, dt=None):
    dt = dt if dt is not None else DT
    key = ("nc", str(dt))
    if key not in _CACHE:
        _CACHE[key] = build_program(dt)
    nc = _CACHE[key]
    in_maps = _prepare_inputs(dt, **inputs)
    res = run_bass_kernel_spmd(nc, in_maps, list(range(NC_)), trace=trace,
                               trace_cores=trace_cores)
    out = np.concatenate([res.results[c]["out_loc"] for c in range(NC_)], axis=0)
    return out.reshape(1, S, HID), res


def kernel(**inputs) -> np.ndarray:
    out, _ = run(inputs, trace=False)
    return out
